# revision 7
# baseline (speedup 1.0000x reference)
"""Trainium2 Bass kernel for a 12-qubit batched PennyLane-style circuit.

Fused formulation (validated in mirror.py):
  Circuit = prod_l [C_l P_l], P_l = tensor of per-sample 1q gates G_{l,q},
  C_l = ring of fixed CRots CR_{l,c} (ctrl c, target c+1 mod 12).
  Rewrite: C_l P_l = CR_{l,11} . [CR_{l,10} G_{l,11}] ... [CR_{l,0} G_{l,1}] . G_{l,0}
  so each CRot fuses with the per-sample gate on its target wire into a
  2-qubit gate that costs the same as a 1q gate (per-sample coefficients on
  both ctrl branches). The wrap CRot CR_{l,11} fuses into layer l+1's G on
  wire 0. Layer 0 (applied to |0..0>) becomes an MPS ladder product state:
  host builds the 64-amplitude prefix over wires 0-5, the device doubles it
  6 times (wires 6-11) with the ladder CRots folded in. Only CR_{3,11}
  remains as a real gate (emitted as a ping-pong gate with identity copies
  on the ctrl=0 half).

Distribution: pure data parallel, 4096 samples -> 8 cores x 512; per core
4 batch tiles of 128 samples (partitions). State = fp32 re/im planes,
batch on partitions, 5 ping-pong buffers (4 bt + 1 spare) so every gate
writes a fresh buffer: no copybacks, chains accumulate in the destination.

Engine split per gate site (gate, bt): ctrl=1 branch -> TensorE as diagonal
matmuls (per-sample coeffs on the diag) accumulating 4 terms in PSUM;
ctrl=0 branch -> 3 chains on DVE + 1 on GpSimd (4-term mult-add chains,
chain start on ScalarE); PSUM evictions + diag builds on ScalarE.
"""

import numpy as np

import concourse.bass as bass
import concourse.bacc as bacc
import concourse.mybir as mybir
from concourse.tile import TileContext
from concourse.bass_utils import run_bass_kernel_spmd

F32 = mybir.dt.float32
F32R = mybir.dt.float32r
ALU = mybir.AluOpType
AF = mybir.ActivationFunctionType

N_QUBITS = 12
N_LAYERS = 4
DIM = 4096
B_FULL = 4096
N_CORES = 8
B_CORE = B_FULL // N_CORES   # 512
NBT = B_CORE // 128          # 4

# coefficient plane order for a 2x2 complex gate [[a,b],[c,d]]
(ARE, AIM, MAIM, BRE, BIM, MBIM,
 CRE, CIM, MCIM, DRE, DIM_, MDIM) = range(12)
NCO = 12

N_GATES = 36                     # layers 1-3, 12 fused gates each
FCO_W = N_GATES * 2 * NCO * NBT  # 3456
KSTEPS = 6                       # device kron steps: wires 6..11
KCO_W = KSTEPS * NCO * NBT       # 288
PREF_W = NBT * 2 * 64            # 512

# per-output chain term tables: (t_out, comp_out) -> 4x (plane, t_in, comp_in)
TERMS = {
    (0, 0): ((ARE, 0, 0), (MAIM, 0, 1), (BRE, 1, 0), (MBIM, 1, 1)),
    (0, 1): ((AIM, 0, 0), (ARE, 0, 1), (BIM, 1, 0), (BRE, 1, 1)),
    (1, 0): ((CRE, 0, 0), (MCIM, 0, 1), (DRE, 1, 0), (MDIM, 1, 1)),
    (1, 1): ((CIM, 0, 0), (CRE, 0, 1), (DIM_, 1, 0), (DRE, 1, 1)),
}
OUTS = ((0, 0), (0, 1), (1, 0), (1, 1))

# ---------------------------------------------------------------------------
# engine plan knobs
# 16 chunk-slots per site: [b=1 outputs o0h0,o0h1,o1h0,...,o3h1, then b=0 ...]
# gpsimd can NOT run scalar_tensor_tensor (neuronxcc rejects it on Pool), so
# chains are DVE-only; gpsimd builds diags / copies / adds instead.
SITE_PLAN = ("pe",) * 8 + ("pe",) + ("dve",) * 7
EVICT_ROT = ("act",)              # psum eviction engines
DIAG_ROT = ("act", "gps")         # diag build engines
CROT_PLAN = ("pe", "dve", "pe", "dve")      # final CRot ctrl=1 outputs
CROT_COPY_ROT = ("act", "gps", "gps", "act")  # final CRot ctrl=0 copies

# ---------------------------------------------------------------------------
# host-side gate algebra
# ---------------------------------------------------------------------------


def _rz(t):
    e = np.exp(-0.5j * t)
    z = np.zeros_like(e)
    return np.stack([np.stack([e, z], -1), np.stack([z, np.conj(e)], -1)], -2)


def _ry(t):
    c = np.cos(t / 2).astype(np.complex128)
    s = np.sin(t / 2).astype(np.complex128)
    return np.stack([np.stack([c, -s], -1), np.stack([s, c], -1)], -2)


def _rx(t):
    c = np.cos(t / 2).astype(np.complex128)
    s = np.sin(t / 2).astype(np.complex128)
    return np.stack([np.stack([c, -1j * s], -1), np.stack([-1j * s, c], -1)], -2)


def _rot(phi, theta, omega):
    # PennyLane Rot = RZ(omega) @ RY(theta) @ RZ(phi)
    return _rz(omega) @ _ry(theta) @ _rz(phi)


def _coef_planes(g):
    """g: [..., 2, 2] complex -> [..., 12] float32 coefficient planes."""
    a, b = g[..., 0, 0], g[..., 0, 1]
    c, d = g[..., 1, 0], g[..., 1, 1]
    cols = [a.real, a.imag, -a.imag, b.real, b.imag, -b.imag,
            c.real, c.imag, -c.imag, d.real, d.imag, -d.imag]
    return np.stack(cols, -1).astype(np.float32)


def _host_gates(x, q_params_rot, q_params_enta):
    x = np.asarray(x, np.float64)
    pr = np.asarray(q_params_rot, np.float64)
    pe = np.asarray(q_params_enta, np.float64)
    enc = np.einsum('qbij,qbjk->qbik',
                    _ry(x[:, 3, :].T),
                    np.einsum('qbij,qbjk->qbik', _rz(x[:, 2, :].T),
                              np.einsum('qbij,qbjk->qbik',
                                        _rx(x[:, 1, :].T), _ry(x[:, 0, :].T))))
    rot = _rot(pr[..., 0], pr[..., 1], pr[..., 2])      # [L,Q,2,2]
    G = np.einsum('lqij,qbjk->lqbik', rot, enc)         # [L,Q,B,2,2]
    U = _rot(pe[..., 0], pe[..., 1], pe[..., 2])        # [L,Q,2,2]
    return G, U


def _host_payload(x, q_params_rot, q_params_enta):
    """Full-batch coefficient arrays: fco [B,3456-layout], kco, pref, cco."""
    B = x.shape[0]
    G, U = _host_gates(x, q_params_rot, q_params_enta)

    # fused gate branch matrices
    fco = np.empty((N_GATES, 2, B, NCO), np.float32)
    for g in range(N_GATES):
        l, q = 1 + g // 12, g % 12
        M0 = G[l, q]
        if q == 0:
            M1 = np.einsum('bij,jk->bik', G[l, 0], U[l - 1, 11])
        else:
            M1 = np.einsum('ij,bjk->bik', U[l, q - 1], G[l, q])
        fco[g, 0] = _coef_planes(M0)
        fco[g, 1] = _coef_planes(M1)

    # kron ladder steps q=6..11: chi_q(b)[j] as a 2x2 "matrix" M[b][j]
    kco = np.empty((KSTEPS, B, NCO), np.float32)
    for k in range(KSTEPS):
        q = 6 + k
        v0 = G[0, q, :, :, 0]                            # [B,2]
        v1 = np.einsum('ij,bj->bi', U[0, q - 1], v0)
        KM = np.empty((B, 2, 2), np.complex128)
        KM[:, 0, :] = v0
        KM[:, 1, :] = v1
        kco[k] = _coef_planes(KM)

    # 64-amplitude prefix over wires 0-5 (ladder CRots folded)
    pref = G[0, 0, :, :, 0]                              # [B,2]
    for q in range(1, 6):
        v0 = G[0, q, :, :, 0]
        v1 = np.einsum('ij,bj->bi', U[0, q - 1], v0)
        w = pref.shape[1]
        new = np.empty((B, 2 * w), np.complex128)
        nv = new.reshape(B, w // 2, 2, 2) if w > 1 else None
        if w == 1:
            raise AssertionError
        old = pref.reshape(B, w // 2, 2)
        for b in (0, 1):
            chi = v0 if b == 0 else v1
            for j in (0, 1):
                nv[:, :, b, j] = old[:, :, b] * chi[:, j][:, None]
        pref = new                                       # [B, 64]

    cco = _coef_planes(U[3, 11])                         # [12]
    return fco, kco, pref, cco


# ---------------------------------------------------------------------------
# bass program
# ---------------------------------------------------------------------------


class _Prog:
    def __init__(self):
        nc = bacc.Bacc("TRN2", target_bir_lowering=False, debug=False)
        self.nc = nc
        self.fco_d = nc.declare_dram_parameter("fco", [128, FCO_W], F32,
                                               isOutput=False)
        self.kco_d = nc.declare_dram_parameter("kco", [128, KCO_W], F32,
                                               isOutput=False)
        self.pref_d = nc.declare_dram_parameter("pref", [128, PREF_W], F32,
                                                isOutput=False)
        self.cco_d = nc.declare_dram_parameter("cco", [128, NCO], F32,
                                               isOutput=False)
        self.idn_d = nc.declare_dram_parameter("ident", [128, 128], F32,
                                               isOutput=False)
        self.z_d = nc.declare_dram_parameter("z", [B_CORE, N_QUBITS], F32,
                                             isOutput=True)
        self._ectr = 0      # eviction engine rotation
        self._dctr = 0      # diag engine rotation
        self._cctr = 0      # chain plan rotation
        with TileContext(nc) as tc:
            self.tc = tc
            with tc.tile_pool(name="main", bufs=1) as pool, \
                    tc.tile_pool(name="dpool", bufs=24) as dpool, \
                    tc.tile_pool(name="psum", bufs=8, space="PSUM") as ppool:
                self.dpool = dpool
                self.ppool = ppool
                # 5 ping-pong plane-pair buffers [re | im], each [128, 8192]
                self.BUF = [pool.tile([128, 2 * DIM], F32R, name=f"st{i}",
                                      tag=f"st{i}") for i in range(5)]
                self.FC = pool.tile([128, FCO_W], F32, tag="fc")
                self.KC = pool.tile([128, KCO_W], F32, tag="kc")
                self.CC = pool.tile([128, NCO], F32, tag="cc")
                self.PS = pool.tile([128, PREF_W], F32, tag="prefs")
                self.I128 = pool.tile([128, 128], F32, tag="ident")
                self.ZT = [pool.tile([128, 16], F32, name=f"z{bt}",
                                     tag=f"z{bt}") for bt in range(NBT)]
                self.cur = [0, 1, 2, 3]
                self.spare = 4

                nc.sync.dma_start(out=self.FC[:], in_=self.fco_d[:])
                nc.sync.dma_start(out=self.KC[:], in_=self.kco_d[:])
                nc.sync.dma_start(out=self.CC[:], in_=self.cco_d[:])
                nc.sync.dma_start(out=self.PS[:], in_=self.pref_d[:])
                nc.sync.dma_start(out=self.I128[:], in_=self.idn_d[:])

                self._emit_circuit()

                for bt in range(NBT):
                    nc.sync.dma_start(
                        out=self.z_d[bt * 128:(bt + 1) * 128, :],
                        in_=self.ZT[bt][:, 0:N_QUBITS])
        nc.compile()

    # ---- AP helpers -----------------------------------------------------

    def plane(self, buf, comp):
        return self.BUF[buf][:, comp * DIM:(comp + 1) * DIM]

    def fsl(self, buf, comp, q, b, t):
        """F-gate slice (wires q-1,q), q in 1..11: ctrl bit=b, target bit=t."""
        p = self.plane(buf, comp)
        if q == 11:
            v = p.rearrange("p (a c t) -> p a c t", c=2, t=2)
            return v[:, :, b, t]                     # [p, 1024] stride 4
        A = 1 << (q - 1)
        R = 1 << (11 - q)
        v = p.rearrange("p (a c t r) -> p a c t r", a=A, c=2, t=2, r=R)
        return v[:, :, b, t, :]                      # [p, A, R]

    def wsl(self, buf, comp, b, t):
        """Wrap-gate slice (wires 11,0): ctrl a11 (LSB)=b, target a0 (MSB)=t."""
        p = self.plane(buf, comp)
        v = p.rearrange("p (t a c) -> p t a c", t=2, c=2)
        return v[:, t, :, b]                         # [p, 1024] stride 2

    def fco(self, g, b, ci, bt):
        idx = ((g * 2 + b) * NCO + ci) * NBT + bt
        return self.FC[:, idx:idx + 1]

    def kco(self, k, ci, bt):
        idx = (k * NCO + ci) * NBT + bt
        return self.KC[:, idx:idx + 1]

    def cco(self, ci):
        return self.CC[:, ci:ci + 1]

    @staticmethod
    def _chunk(view, idx, csz):
        """csz-wide column chunk of an AP shaped [128, w] or [128, n, s]."""
        shp = view.shape[1:]
        if len(shp) == 1:
            return view[:, idx * csz:(idx + 1) * csz]
        n, s = shp
        if s >= csz:
            m = s // csz
            return view[:, idx // m, (idx % m) * csz:(idx % m + 1) * csz]
        na = csz // s
        return view[:, idx * na:(idx + 1) * na, :]

    def _eng(self, name):
        return {"dve": self.nc.vector, "gps": self.nc.gpsimd}[name]

    # ---- gate emission --------------------------------------------------

    def _build_diags(self, co):
        nc = self.nc
        D = {}
        for ci in range(NCO):
            d = self.dpool.tile([128, 128], F32R, name="dg", tag="dg")
            e = DIAG_ROT[self._dctr % len(DIAG_ROT)]
            self._dctr += 1
            if e == "act":
                nc.scalar.activation(d[:], self.I128[:], AF.Copy,
                                     scale=co(ci))
            else:
                self._eng(e).tensor_scalar(d[:], self.I128[:], co(ci),
                                           None, ALU.mult)
            D[ci] = d
        return D

    def _pe_out(self, dst, srcs, planes, D, nchunks, csz=512):
        """One output slice via TensorE diag matmuls, chunked into PSUM."""
        nc = self.nc
        for h in range(nchunks):
            ps = self.ppool.tile([128, csz], F32, name="ps", tag="ps")
            for k in range(4):
                nc.tensor.matmul(out=ps[:], lhsT=D[planes[k]][:],
                                 rhs=self._chunk(srcs[k], h, csz),
                                 start=(k == 0), stop=(k == 3))
            dc = self._chunk(dst, h, csz)
            src = ps[:]
            if len(dc.shape) > 2:
                src = src.rearrange("p (a r) -> p a r", r=dc.shape[-1])
            e = EVICT_ROT[self._ectr % len(EVICT_ROT)]
            self._ectr += 1
            if e == "act":
                nc.scalar.copy(dc, src)
            else:
                self._eng(e).tensor_copy(out=dc, in_=src)

    def _emit_site(self, sl_src, sl_dst, co):
        """Emit one fused-gate site: slices are dicts (b,t,comp)->AP.
        Work units follow SITE_PLAN (16 chunk slots); adjacent same-engine
        chunks of one output are coalesced into full-width ops."""
        nc = self.nc
        pe_units = []   # (branch, dst-view, src-views[4], planes, nchunks)
        chains = []     # (engine, dst, srcs[4], coefs[4])
        pe_branches = set()
        for bi, b in enumerate((1, 0)):
            for oi, (t, c) in enumerate(OUTS):
                dst = sl_dst[(b, t, c)]
                terms = TERMS[(t, c)]
                srcs = [sl_src[(b, ti, ci)] for (_, ti, ci) in terms]
                planes = [pl for (pl, _, _) in terms]
                e0 = SITE_PLAN[bi * 8 + oi * 2]
                e1 = SITE_PLAN[bi * 8 + oi * 2 + 1]
                if e0 == e1:
                    if e0 == "pe":
                        pe_branches.add(b)
                        pe_units.append((b, dst, srcs, planes, 2))
                    else:
                        chains.append((e0, dst, srcs,
                                       [co(b, pl) for pl in planes]))
                else:
                    for h, e in ((0, e0), (1, e1)):
                        dc = self._chunk2(dst, h)
                        sc = [self._chunk2(s, h) for s in srcs]
                        if e == "pe":
                            pe_branches.add(b)
                            pe_units.append((b, dc, sc, planes, 1))
                        else:
                            chains.append((e, dc, sc,
                                           [co(b, pl) for pl in planes]))
        # emission order: chain starts (ScalarE) first so DVE can begin,
        # then diags + matmuls, then STT rounds
        for (e, dst, srcs, coefs) in chains:
            nc.scalar.activation(dst, srcs[0], AF.Copy, scale=coefs[0])
        D = {b: self._build_diags(lambda ci: co(b, ci)) for b in pe_branches}
        for (b, dst, srcs, planes, nch) in pe_units:
            self._pe_out(dst, srcs, planes, D[b], nchunks=nch)
        for k in range(1, 4):
            for (e, dst, srcs, coefs) in chains:
                self._eng(e).scalar_tensor_tensor(dst, srcs[k], coefs[k],
                                                  dst, ALU.mult, ALU.add)

    def _chunk2(self, view, h):
        """half-split a 1024-col slice into 512-col chunks"""
        return self._chunk(view, h, 512)

    def _emit_chains(self, chains):
        """chains: list of (eng, dst, srcs[4], coefs[4]); starts on ScalarE,
        then STT accumulation interleaved round-robin per engine."""
        nc = self.nc
        for (e, dst, srcs, coefs) in chains:
            nc.scalar.activation(dst, srcs[0], AF.Copy, scale=coefs[0])
        for k in range(1, 4):
            for (e, dst, srcs, coefs) in chains:
                self._eng(e).scalar_tensor_tensor(dst, srcs[k], coefs[k],
                                                  dst, ALU.mult, ALU.add)

    def _fused_gate(self, g, q, bt):
        src, dst = self.cur[bt], self.spare
        sl_src, sl_dst = {}, {}
        for b in (0, 1):
            for t in (0, 1):
                for c in (0, 1):
                    if q == 0:
                        sl_src[(b, t, c)] = self.wsl(src, c, b, t)
                        sl_dst[(b, t, c)] = self.wsl(dst, c, b, t)
                    else:
                        sl_src[(b, t, c)] = self.fsl(src, c, q, b, t)
                        sl_dst[(b, t, c)] = self.fsl(dst, c, q, b, t)
        co = lambda b, ci: self.fco(g, b, ci, bt)
        self._emit_site(sl_src, sl_dst, co)
        self.spare, self.cur[bt] = self.cur[bt], self.spare

    # ---- layer 0: prefix load + kron ladder ------------------------------

    def _load_prefix(self, bt):
        nc = self.nc
        dst = self.BUF[self.cur[bt]].rearrange("p (c n) -> p c n", c=2)
        src = self.PS[:, bt * 128:(bt + 1) * 128].rearrange(
            "p (c n) -> p c n", c=2)
        nc.scalar.copy(dst[:, :, 0:64], src)

    def _kron_step(self, k, bt):
        """Double width w -> 2w appending wire q=6+k, ladder CRot folded."""
        nc = self.nc
        w = 64 << k
        src, dst = self.cur[bt], self.spare
        units = []
        for b in (0, 1):
            for j in (0, 1):
                pl = 3 * (2 * b + j)     # re plane of entry [b][j]
                for comp in (0, 1):
                    old_re = self.plane(src, 0)[:, 0:w].rearrange(
                        "p (a pb) -> p a pb", pb=2)[:, :, b]
                    old_im = self.plane(src, 1)[:, 0:w].rearrange(
                        "p (a pb) -> p a pb", pb=2)[:, :, b]
                    d = self.plane(dst, comp)[:, 0:2 * w].rearrange(
                        "p (a pb j) -> p a pb j", pb=2, j=2)[:, :, b, j]
                    if comp == 0:
                        # re = old_re*chi_re + old_im*(-chi_im)
                        units.append((d, old_re, self.kco(k, pl, bt),
                                      old_im, self.kco(k, pl + 2, bt)))
                    else:
                        # im = old_re*chi_im + old_im*chi_re
                        units.append((d, old_re, self.kco(k, pl + 1, bt),
                                      old_im, self.kco(k, pl, bt)))
        for (d, s0, c0, s1, c1) in units:
            nc.scalar.activation(d, s0, AF.Copy, scale=c0)
        for (d, s0, c0, s1, c1) in units:
            nc.vector.scalar_tensor_tensor(d, s1, c1, d, ALU.mult, ALU.add)
        self.spare, self.cur[bt] = self.cur[bt], self.spare

    # ---- final CRot ------------------------------------------------------

    def _final_crot(self, bt, D):
        """CR_{3,11}: ctrl=1 gets U, ctrl=0 identity copies; ping-pong."""
        nc = self.nc
        src, dst = self.cur[bt], self.spare
        # ctrl=0: plain copies
        for i, (t, c) in enumerate(OUTS):
            s = self.wsl(src, c, 0, t)
            d = self.wsl(dst, c, 0, t)
            e = CROT_COPY_ROT[i % len(CROT_COPY_ROT)]
            if e == "act":
                nc.scalar.copy(d, s)
            elif e == "dve":
                nc.vector.tensor_copy(out=d, in_=s)
            else:
                nc.gpsimd.tensor_copy(out=d, in_=s)
        # ctrl=1: gate with fixed broadcast coeffs
        chains = []
        for oi, (t, c) in enumerate(OUTS):
            dst_ap = self.wsl(dst, c, 1, t)
            terms = TERMS[(t, c)]
            srcs = [self.wsl(src, ci, 1, ti) for (_, ti, ci) in terms]
            planes = [pl for (pl, _, _) in terms]
            e = CROT_PLAN[oi % len(CROT_PLAN)]
            if e == "pe":
                self._pe_out(dst_ap, srcs, planes, D, nchunks=2)
            else:
                chains.append((e, dst_ap, srcs,
                               [self.cco(pl) for pl in planes]))
        self._emit_chains(chains)
        self.spare, self.cur[bt] = self.cur[bt], self.spare

    # ---- observables -----------------------------------------------------

    def _observables(self, bt):
        """probs overwrite the re plane in place; im plane is scratch."""
        nc = self.nc
        buf = self.cur[bt]
        re = self.plane(buf, 0)
        im = self.plane(buf, 1)
        for h in range(4):
            sl = slice(h * 1024, (h + 1) * 1024)
            nc.scalar.activation(re[:, sl], re[:, sl], AF.Square)
            nc.scalar.activation(im[:, sl], im[:, sl], AF.Square)
            nc.vector.tensor_tensor(re[:, sl], re[:, sl], im[:, sl], ALU.add)
        w = DIM
        for q in range(N_QUBITS):
            h = w // 2
            lo, hi = re[:, 0:h], re[:, h:w]
            scr = im[:, 0:1024]
            if h > 1024:
                for kk in range(h // 1024):
                    sk = slice(kk * 1024, (kk + 1) * 1024)
                    nc.vector.tensor_tensor(scr, lo[:, sk], hi[:, sk],
                                            ALU.subtract)
                    nc.vector.tensor_reduce(
                        out=self.ZT[bt][:, 12 + kk:13 + kk], in_=scr,
                        op=ALU.add, axis=mybir.AxisListType.X)
                nc.vector.tensor_tensor(self.ZT[bt][:, q:q + 1],
                                        self.ZT[bt][:, 12:13],
                                        self.ZT[bt][:, 13:14], ALU.add)
            else:
                nc.vector.tensor_tensor(scr[:, 0:h], lo, hi, ALU.subtract)
                nc.vector.tensor_reduce(out=self.ZT[bt][:, q:q + 1],
                                        in_=scr[:, 0:h], op=ALU.add,
                                        axis=mybir.AxisListType.X)
            if q < N_QUBITS - 1:
                for kk in range(max(1, h // 1024)):
                    sk = slice(kk * 1024, min((kk + 1) * 1024, h))
                    eng = nc.vector if kk % 2 == 0 else nc.gpsimd
                    eng.tensor_tensor(lo[:, sk], lo[:, sk], hi[:, sk],
                                      ALU.add)
            w = h

    # ---- top level -------------------------------------------------------

    def _emit_circuit(self):
        for bt in range(NBT):
            self._load_prefix(bt)
        for k in range(KSTEPS):
            for bt in range(NBT):
                self._kron_step(k, bt)
        for g in range(N_GATES):
            q = g % 12
            for bt in range(NBT):
                self._fused_gate(g, q, bt)
        Dc = self._build_diags(lambda ci: self.cco(ci))
        for bt in range(NBT):
            self._final_crot(bt, Dc)
        for bt in range(NBT):
            self._observables(bt)


_PROG_CACHE = None


def _get_prog():
    global _PROG_CACHE
    if _PROG_CACHE is None:
        _PROG_CACHE = _Prog()
    return _PROG_CACHE


def _run(inputs, trace=False):
    x = np.asarray(inputs["x"], np.float32)
    fco, kco, pref, cco = _host_payload(
        x, inputs["q_params_rot"], inputs["q_params_enta"])
    # fco: [G,2,B,12] -> per-core tile [128, ((g*2+b)*12+ci)*4+bt]
    cco_tile = np.broadcast_to(cco.reshape(1, NCO), (128, NCO)).copy()
    ident = np.eye(128, dtype=np.float32)
    in_maps = []
    for core in range(N_CORES):
        lo = core * B_CORE
        f = fco[:, :, lo:lo + B_CORE, :]                  # [G,2,512,12]
        f = f.reshape(N_GATES, 2, NBT, 128, NCO)
        f = np.ascontiguousarray(np.transpose(f, (3, 0, 1, 4, 2)))
        k = kco[:, lo:lo + B_CORE, :].reshape(KSTEPS, NBT, 128, NCO)
        k = np.ascontiguousarray(np.transpose(k, (2, 0, 3, 1)))
        p = pref[lo:lo + B_CORE].reshape(NBT, 128, 64)    # complex
        pr = np.empty((128, NBT, 2, 64), np.float32)
        pr[:, :, 0, :] = np.moveaxis(p.real, 1, 0)
        pr[:, :, 1, :] = np.moveaxis(p.imag, 1, 0)
        in_maps.append({
            "fco": f.reshape(128, FCO_W),
            "kco": k.reshape(128, KCO_W),
            "pref": np.ascontiguousarray(pr.reshape(128, PREF_W)),
            "cco": cco_tile,
            "ident": ident,
        })
    prog = _get_prog()
    res = run_bass_kernel_spmd(prog.nc, in_maps, list(range(N_CORES)),
                               trace=trace)
    z = np.concatenate([res.results[c]["z"] for c in range(N_CORES)], axis=0)
    return z.astype(np.float32), res


def kernel(**inputs):
    z, _ = _run(inputs, trace=False)
    return z


# revision 16
# speedup vs baseline: 1.4046x; 1.4046x over previous
"""Trainium2 Bass kernel for a 12-qubit batched PennyLane-style circuit.

Fused formulation (validated in mirror.py):
  Circuit = prod_l [C_l P_l], P_l = tensor of per-sample 1q gates G_{l,q},
  C_l = ring of fixed CRots CR_{l,c} (ctrl c, target c+1 mod 12).
  Rewrite: C_l P_l = CR_{l,11} . [CR_{l,10} G_{l,11}] ... [CR_{l,0} G_{l,1}] . G_{l,0}
  so each CRot fuses with the per-sample gate on its target wire into a
  2-qubit gate that costs the same as a 1q gate (per-sample coefficients on
  both ctrl branches). The wrap CRot CR_{l,11} fuses into layer l+1's G on
  wire 0. Layer 0 (applied to |0..0>) becomes an MPS ladder product state:
  host builds the 64-amplitude prefix over wires 0-5, the device doubles it
  6 times (wires 6-11) with the ladder CRots folded in. Only CR_{3,11}
  remains as a real gate (emitted as a ping-pong gate with identity copies
  on the ctrl=0 half).

Distribution: pure data parallel, 4096 samples -> 8 cores x 512; per core
4 batch tiles of 128 samples (partitions). State = fp32 re/im planes,
batch on partitions, 5 ping-pong buffers (4 bt + 1 spare) so every gate
writes a fresh buffer: no copybacks, chains accumulate in the destination.

Engine split per gate site (gate, bt): ctrl=1 branch -> TensorE as diagonal
matmuls (per-sample coeffs on the diag) accumulating 4 terms in PSUM;
ctrl=0 branch -> 3 chains on DVE + 1 on GpSimd (4-term mult-add chains,
chain start on ScalarE); PSUM evictions + diag builds on ScalarE.
"""

import numpy as np

import concourse.bass as bass
import concourse.bacc as bacc
import concourse.mybir as mybir
from concourse.tile import TileContext
from concourse.bass_utils import run_bass_kernel_spmd

F32 = mybir.dt.float32
F32R = mybir.dt.float32r
ALU = mybir.AluOpType
AF = mybir.ActivationFunctionType

N_QUBITS = 12
N_LAYERS = 4
DIM = 4096
B_FULL = 4096
N_CORES = 8
B_CORE = B_FULL // N_CORES   # 512
NBT = B_CORE // 128          # 4

# coefficient plane order for a 2x2 complex gate [[a,b],[c,d]]
(ARE, AIM, MAIM, BRE, BIM, MBIM,
 CRE, CIM, MCIM, DRE, DIM_, MDIM) = range(12)
NCO = 12

N_GATES = 36                     # layers 1-3, 12 fused gates each
FCO_W = N_GATES * 2 * NCO * NBT  # 3456
KSTEPS = 6                       # device kron steps: wires 6..11
KCO_W = KSTEPS * NCO * NBT       # 288
PREF_W = NBT * 2 * 64            # 512

# per-output chain term tables: (t_out, comp_out) -> 4x (plane, t_in, comp_in)
TERMS = {
    (0, 0): ((ARE, 0, 0), (MAIM, 0, 1), (BRE, 1, 0), (MBIM, 1, 1)),
    (0, 1): ((AIM, 0, 0), (ARE, 0, 1), (BIM, 1, 0), (BRE, 1, 1)),
    (1, 0): ((CRE, 0, 0), (MCIM, 0, 1), (DRE, 1, 0), (MDIM, 1, 1)),
    (1, 1): ((CIM, 0, 0), (CRE, 0, 1), (DIM_, 1, 0), (DRE, 1, 1)),
}
OUTS = ((0, 0), (0, 1), (1, 0), (1, 1))

# ---------------------------------------------------------------------------
# engine plan knobs
# 16 chunk-slots per site: [b=1 outputs o0h0,o0h1,o1h0,...,o3h1, then b=0 ...]
# gpsimd can NOT run scalar_tensor_tensor (neuronxcc rejects it on Pool), so
# chains are DVE-only; gpsimd builds diags / copies / adds instead.
SITE_PLAN = ("pe",) * 8 + ("pe",) + ("dve",) * 7
EVICT_ROT = ("act",)              # psum eviction engines
DIAG_ROT = ("act",)               # diag build engines
CROT_PLAN = ("pe", "dve", "pe", "dve")      # final CRot ctrl=1 outputs
CROT_COPY_ROT = ("act", "dve", "act", "dve")  # final CRot ctrl=0 copies

# ---------------------------------------------------------------------------
# host-side gate algebra
# ---------------------------------------------------------------------------


def _rz(t):
    e = np.exp(-0.5j * t)
    z = np.zeros_like(e)
    return np.stack([np.stack([e, z], -1), np.stack([z, np.conj(e)], -1)], -2)


def _ry(t):
    c = np.cos(t / 2).astype(np.complex128)
    s = np.sin(t / 2).astype(np.complex128)
    return np.stack([np.stack([c, -s], -1), np.stack([s, c], -1)], -2)


def _rx(t):
    c = np.cos(t / 2).astype(np.complex128)
    s = np.sin(t / 2).astype(np.complex128)
    return np.stack([np.stack([c, -1j * s], -1), np.stack([-1j * s, c], -1)], -2)


def _rot(phi, theta, omega):
    # PennyLane Rot = RZ(omega) @ RY(theta) @ RZ(phi)
    return _rz(omega) @ _ry(theta) @ _rz(phi)


def _coef_planes(g):
    """g: [..., 2, 2] complex -> [..., 12] float32 coefficient planes."""
    a, b = g[..., 0, 0], g[..., 0, 1]
    c, d = g[..., 1, 0], g[..., 1, 1]
    cols = [a.real, a.imag, -a.imag, b.real, b.imag, -b.imag,
            c.real, c.imag, -c.imag, d.real, d.imag, -d.imag]
    return np.stack(cols, -1).astype(np.float32)


def _host_gates(x, q_params_rot, q_params_enta):
    x = np.asarray(x, np.float64)
    pr = np.asarray(q_params_rot, np.float64)
    pe = np.asarray(q_params_enta, np.float64)
    enc = np.einsum('qbij,qbjk->qbik',
                    _ry(x[:, 3, :].T),
                    np.einsum('qbij,qbjk->qbik', _rz(x[:, 2, :].T),
                              np.einsum('qbij,qbjk->qbik',
                                        _rx(x[:, 1, :].T), _ry(x[:, 0, :].T))))
    rot = _rot(pr[..., 0], pr[..., 1], pr[..., 2])      # [L,Q,2,2]
    G = np.einsum('lqij,qbjk->lqbik', rot, enc)         # [L,Q,B,2,2]
    U = _rot(pe[..., 0], pe[..., 1], pe[..., 2])        # [L,Q,2,2]
    return G, U


def _host_payload(x, q_params_rot, q_params_enta):
    """Full-batch coefficient arrays: fco [B,3456-layout], kco, pref, cco."""
    B = x.shape[0]
    G, U = _host_gates(x, q_params_rot, q_params_enta)

    # fused gate branch matrices
    fco = np.empty((N_GATES, 2, B, NCO), np.float32)
    for g in range(N_GATES):
        l, q = 1 + g // 12, g % 12
        M0 = G[l, q]
        if q == 0:
            M1 = np.einsum('bij,jk->bik', G[l, 0], U[l - 1, 11])
        else:
            M1 = np.einsum('ij,bjk->bik', U[l, q - 1], G[l, q])
        fco[g, 0] = _coef_planes(M0)
        fco[g, 1] = _coef_planes(M1)

    # kron ladder steps q=6..11: chi_q(b)[j] as a 2x2 "matrix" M[b][j]
    kco = np.empty((KSTEPS, B, NCO), np.float32)
    for k in range(KSTEPS):
        q = 6 + k
        v0 = G[0, q, :, :, 0]                            # [B,2]
        v1 = np.einsum('ij,bj->bi', U[0, q - 1], v0)
        KM = np.empty((B, 2, 2), np.complex128)
        KM[:, 0, :] = v0
        KM[:, 1, :] = v1
        kco[k] = _coef_planes(KM)

    # 64-amplitude prefix over wires 0-5 (ladder CRots folded)
    pref = G[0, 0, :, :, 0]                              # [B,2]
    for q in range(1, 6):
        v0 = G[0, q, :, :, 0]
        v1 = np.einsum('ij,bj->bi', U[0, q - 1], v0)
        w = pref.shape[1]
        new = np.empty((B, 2 * w), np.complex128)
        nv = new.reshape(B, w // 2, 2, 2) if w > 1 else None
        if w == 1:
            raise AssertionError
        old = pref.reshape(B, w // 2, 2)
        for b in (0, 1):
            chi = v0 if b == 0 else v1
            for j in (0, 1):
                nv[:, :, b, j] = old[:, :, b] * chi[:, j][:, None]
        pref = new                                       # [B, 64]

    cco = _coef_planes(U[3, 11])                         # [12]
    return fco, kco, pref, cco


# ---------------------------------------------------------------------------
# bass program
# ---------------------------------------------------------------------------


class _Prog:
    def __init__(self):
        nc = bacc.Bacc("TRN2", target_bir_lowering=False, debug=False)
        self.nc = nc
        self.fco_d = nc.declare_dram_parameter("fco", [128, FCO_W], F32,
                                               isOutput=False)
        self.kco_d = nc.declare_dram_parameter("kco", [128, KCO_W], F32,
                                               isOutput=False)
        self.pref_d = nc.declare_dram_parameter("pref", [128, PREF_W], F32,
                                                isOutput=False)
        self.cco_d = nc.declare_dram_parameter("cco", [128, NCO], F32,
                                               isOutput=False)
        self.idn_d = nc.declare_dram_parameter("ident", [128, 128], F32,
                                               isOutput=False)
        self.z_d = nc.declare_dram_parameter("z", [B_CORE, N_QUBITS], F32,
                                             isOutput=True)
        self._ectr = 0      # eviction engine rotation
        self._dctr = 0      # diag engine rotation
        self._cctr = 0      # chain plan rotation
        with TileContext(nc) as tc:
            self.tc = tc
            with tc.tile_pool(name="main", bufs=1) as pool, \
                    tc.tile_pool(name="dpool", bufs=24) as dpool, \
                    tc.tile_pool(name="psum", bufs=8, space="PSUM") as ppool:
                self.dpool = dpool
                self.ppool = ppool
                # 5 ping-pong plane-pair buffers [re | im], each [128, 8192]
                self.BUF = [pool.tile([128, 2 * DIM], F32R, name=f"st{i}",
                                      tag=f"st{i}") for i in range(5)]
                self.FC = pool.tile([128, FCO_W], F32, tag="fc")
                self.KC = pool.tile([128, KCO_W], F32, tag="kc")
                self.CC = pool.tile([128, NCO], F32, tag="cc")
                self.PS = pool.tile([128, PREF_W], F32, tag="prefs")
                self.I128 = pool.tile([128, 128], F32, tag="ident")
                self.ZT = [pool.tile([128, 16], F32, name=f"z{bt}",
                                     tag=f"z{bt}") for bt in range(NBT)]
                self.SCR = [pool.tile([128, 2048], F32, name=f"scr{i}",
                                      tag=f"scr{i}") for i in range(2)]
                self.cur = [0, 1, 2, 3]
                self.spare = 4

                nc.sync.dma_start(out=self.FC[:], in_=self.fco_d[:])
                nc.sync.dma_start(out=self.KC[:], in_=self.kco_d[:])
                nc.sync.dma_start(out=self.CC[:], in_=self.cco_d[:])
                nc.sync.dma_start(out=self.PS[:], in_=self.pref_d[:])
                nc.sync.dma_start(out=self.I128[:], in_=self.idn_d[:])

                self._emit_circuit()

                for bt in range(NBT):
                    nc.sync.dma_start(
                        out=self.z_d[bt * 128:(bt + 1) * 128, :],
                        in_=self.ZT[bt][:, 0:N_QUBITS])
        nc.compile()

    # ---- AP helpers -----------------------------------------------------

    def plane(self, buf, comp):
        return self.BUF[buf][:, comp * DIM:(comp + 1) * DIM]

    def fsl(self, buf, comp, q, b, t):
        """F-gate slice (wires q-1,q), q in 1..11: ctrl bit=b, target bit=t."""
        p = self.plane(buf, comp)
        if q == 11:
            v = p.rearrange("p (a c t) -> p a c t", c=2, t=2)
            return v[:, :, b, t]                     # [p, 1024] stride 4
        A = 1 << (q - 1)
        R = 1 << (11 - q)
        v = p.rearrange("p (a c t r) -> p a c t r", a=A, c=2, t=2, r=R)
        return v[:, :, b, t, :]                      # [p, A, R]

    def wsl(self, buf, comp, b, t):
        """Wrap-gate slice (wires 11,0): ctrl a11 (LSB)=b, target a0 (MSB)=t."""
        p = self.plane(buf, comp)
        v = p.rearrange("p (t a c) -> p t a c", t=2, c=2)
        return v[:, t, :, b]                         # [p, 1024] stride 2

    def fco(self, g, b, ci, bt):
        idx = ((g * 2 + b) * NCO + ci) * NBT + bt
        return self.FC[:, idx:idx + 1]

    def kco(self, k, ci, bt):
        idx = (k * NCO + ci) * NBT + bt
        return self.KC[:, idx:idx + 1]

    def cco(self, ci):
        return self.CC[:, ci:ci + 1]

    @staticmethod
    def _chunk(view, idx, csz):
        """csz-wide column chunk of an AP shaped [128, w] or [128, n, s]."""
        shp = view.shape[1:]
        if len(shp) == 1:
            return view[:, idx * csz:(idx + 1) * csz]
        n, s = shp
        if s >= csz:
            m = s // csz
            return view[:, idx // m, (idx % m) * csz:(idx % m + 1) * csz]
        na = csz // s
        return view[:, idx * na:(idx + 1) * na, :]

    def _eng(self, name):
        return {"dve": self.nc.vector, "gps": self.nc.gpsimd}[name]

    # ---- gate emission --------------------------------------------------

    def _build_diags(self, co):
        nc = self.nc
        D = {}
        for ci in range(NCO):
            d = self.dpool.tile([128, 128], F32R, name="dg", tag="dg")
            e = DIAG_ROT[self._dctr % len(DIAG_ROT)]
            self._dctr += 1
            if e == "act":
                nc.scalar.activation(d[:], self.I128[:], AF.Copy,
                                     scale=co(ci))
            else:
                self._eng(e).tensor_scalar(d[:], self.I128[:], co(ci),
                                           None, ALU.mult)
            D[ci] = d
        return D

    def _pe_out(self, dst, srcs, planes, D, nchunks, csz=512):
        """One output slice via TensorE diag matmuls, chunked into PSUM."""
        nc = self.nc
        for h in range(nchunks):
            ps = self.ppool.tile([128, csz], F32, name="ps", tag="ps")
            for k in range(4):
                nc.tensor.matmul(out=ps[:], lhsT=D[planes[k]][:],
                                 rhs=self._chunk(srcs[k], h, csz),
                                 start=(k == 0), stop=(k == 3))
            dc = self._chunk(dst, h, csz)
            src = ps[:]
            if len(dc.shape) > 2:
                src = src.rearrange("p (a r) -> p a r", r=dc.shape[-1])
            e = EVICT_ROT[self._ectr % len(EVICT_ROT)]
            self._ectr += 1
            if e == "act":
                nc.scalar.copy(dc, src)
            else:
                self._eng(e).tensor_copy(out=dc, in_=src)

    def _emit_site(self, sl_src, sl_dst, co):
        """Emit one fused-gate site: slices are dicts (b,t,comp)->AP.
        Work units follow SITE_PLAN (16 chunk slots); adjacent same-engine
        chunks of one output are coalesced into full-width ops."""
        nc = self.nc
        pe_units = []   # (branch, dst-view, src-views[4], planes, nchunks)
        chains = []     # (engine, dst, srcs[4], coefs[4])
        pe_branches = set()
        for bi, b in enumerate((1, 0)):
            for oi, (t, c) in enumerate(OUTS):
                dst = sl_dst[(b, t, c)]
                terms = TERMS[(t, c)]
                srcs = [sl_src[(b, ti, ci)] for (_, ti, ci) in terms]
                planes = [pl for (pl, _, _) in terms]
                e0 = SITE_PLAN[bi * 8 + oi * 2]
                e1 = SITE_PLAN[bi * 8 + oi * 2 + 1]
                if e0 == e1:
                    if e0 == "pe":
                        pe_branches.add(b)
                        pe_units.append((b, dst, srcs, planes, 2))
                    else:
                        chains.append((e0, dst, srcs,
                                       [co(b, pl) for pl in planes]))
                else:
                    for h, e in ((0, e0), (1, e1)):
                        dc = self._chunk2(dst, h)
                        sc = [self._chunk2(s, h) for s in srcs]
                        if e == "pe":
                            pe_branches.add(b)
                            pe_units.append((b, dc, sc, planes, 1))
                        else:
                            chains.append((e, dc, sc,
                                           [co(b, pl) for pl in planes]))
        # emission order: chain starts (ScalarE) first so DVE can begin,
        # then diags + matmuls, then STT rounds
        for (e, dst, srcs, coefs) in chains:
            nc.scalar.activation(dst, srcs[0], AF.Copy, scale=coefs[0])
        D = {b: self._build_diags(lambda ci: co(b, ci)) for b in pe_branches}
        for (b, dst, srcs, planes, nch) in pe_units:
            self._pe_out(dst, srcs, planes, D[b], nchunks=nch)
        for k in range(1, 4):
            for (e, dst, srcs, coefs) in chains:
                self._eng(e).scalar_tensor_tensor(dst, srcs[k], coefs[k],
                                                  dst, ALU.mult, ALU.add)

    def _chunk2(self, view, h):
        """half-split a 1024-col slice into 512-col chunks"""
        return self._chunk(view, h, 512)

    def _emit_chains(self, chains):
        """chains: list of (eng, dst, srcs[4], coefs[4]); starts on ScalarE,
        then STT accumulation interleaved round-robin per engine."""
        nc = self.nc
        for (e, dst, srcs, coefs) in chains:
            nc.scalar.activation(dst, srcs[0], AF.Copy, scale=coefs[0])
        for k in range(1, 4):
            for (e, dst, srcs, coefs) in chains:
                self._eng(e).scalar_tensor_tensor(dst, srcs[k], coefs[k],
                                                  dst, ALU.mult, ALU.add)

    def _fused_gate(self, g, q, bt):
        src, dst = self.cur[bt], self.spare
        sl_src, sl_dst = {}, {}
        for b in (0, 1):
            for t in (0, 1):
                for c in (0, 1):
                    if q == 0:
                        sl_src[(b, t, c)] = self.wsl(src, c, b, t)
                        sl_dst[(b, t, c)] = self.wsl(dst, c, b, t)
                    else:
                        sl_src[(b, t, c)] = self.fsl(src, c, q, b, t)
                        sl_dst[(b, t, c)] = self.fsl(dst, c, q, b, t)
        co = lambda b, ci: self.fco(g, b, ci, bt)
        self._emit_site(sl_src, sl_dst, co)
        self.spare, self.cur[bt] = self.cur[bt], self.spare

    # ---- layer 0: prefix load + kron ladder ------------------------------

    def _load_prefix(self, bt):
        nc = self.nc
        dst = self.BUF[self.cur[bt]].rearrange("p (c n) -> p c n", c=2)
        src = self.PS[:, bt * 128:(bt + 1) * 128].rearrange(
            "p (c n) -> p c n", c=2)
        nc.scalar.copy(dst[:, :, 0:64], src)

    def _kron_step(self, k, bt):
        """Double width w -> 2w appending wire q=6+k, ladder CRot folded."""
        nc = self.nc
        w = 64 << k
        src, dst = self.cur[bt], self.spare
        units = []
        for b in (0, 1):
            for j in (0, 1):
                pl = 3 * (2 * b + j)     # re plane of entry [b][j]
                for comp in (0, 1):
                    old_re = self.plane(src, 0)[:, 0:w].rearrange(
                        "p (a pb) -> p a pb", pb=2)[:, :, b]
                    old_im = self.plane(src, 1)[:, 0:w].rearrange(
                        "p (a pb) -> p a pb", pb=2)[:, :, b]
                    d = self.plane(dst, comp)[:, 0:2 * w].rearrange(
                        "p (a pb j) -> p a pb j", pb=2, j=2)[:, :, b, j]
                    if comp == 0:
                        # re = old_re*chi_re + old_im*(-chi_im)
                        units.append((d, old_re, self.kco(k, pl, bt),
                                      old_im, self.kco(k, pl + 2, bt)))
                    else:
                        # im = old_re*chi_im + old_im*chi_re
                        units.append((d, old_re, self.kco(k, pl + 1, bt),
                                      old_im, self.kco(k, pl, bt)))
        for (d, s0, c0, s1, c1) in units:
            nc.scalar.activation(d, s0, AF.Copy, scale=c0)
        for (d, s0, c0, s1, c1) in units:
            nc.vector.scalar_tensor_tensor(d, s1, c1, d, ALU.mult, ALU.add)
        self.spare, self.cur[bt] = self.cur[bt], self.spare

    # ---- final CRot ------------------------------------------------------

    def _final_crot(self, bt, D):
        """CR_{3,11}: ctrl=1 gets U, ctrl=0 identity copies; ping-pong."""
        nc = self.nc
        src, dst = self.cur[bt], self.spare
        # ctrl=0: plain copies
        for i, (t, c) in enumerate(OUTS):
            s = self.wsl(src, c, 0, t)
            d = self.wsl(dst, c, 0, t)
            e = CROT_COPY_ROT[i % len(CROT_COPY_ROT)]
            if e == "act":
                nc.scalar.copy(d, s)
            elif e == "dve":
                nc.vector.tensor_copy(out=d, in_=s)
            else:
                nc.gpsimd.tensor_copy(out=d, in_=s)
        # ctrl=1: gate with fixed broadcast coeffs
        chains = []
        for oi, (t, c) in enumerate(OUTS):
            dst_ap = self.wsl(dst, c, 1, t)
            terms = TERMS[(t, c)]
            srcs = [self.wsl(src, ci, 1, ti) for (_, ti, ci) in terms]
            planes = [pl for (pl, _, _) in terms]
            e = CROT_PLAN[oi % len(CROT_PLAN)]
            if e == "pe":
                self._pe_out(dst_ap, srcs, planes, D, nchunks=2)
            else:
                chains.append((e, dst_ap, srcs,
                               [self.cco(pl) for pl in planes]))
        self._emit_chains(chains)
        self.spare, self.cur[bt] = self.cur[bt], self.spare

    # ---- observables -----------------------------------------------------

    def _observables(self, bt):
        """probs overwrite the re plane in place; im plane is scratch."""
        nc = self.nc
        buf = self.cur[bt]
        re = self.plane(buf, 0)
        im = self.plane(buf, 1)
        for h in range(4):
            sl = slice(h * 1024, (h + 1) * 1024)
            nc.scalar.activation(re[:, sl], re[:, sl], AF.Square)
            nc.scalar.activation(im[:, sl], im[:, sl], AF.Square)
            nc.vector.tensor_tensor(re[:, sl], re[:, sl], im[:, sl], ALU.add)
        w = DIM
        for q in range(N_QUBITS):
            h = w // 2
            lo, hi = re[:, 0:h], re[:, h:w]
            # (lo - hi) into f32 scratch, then reduce into ZT
            scr = self.SCR[bt % 2][:, 0:h]
            nc.vector.tensor_tensor(scr, lo, hi, ALU.subtract)
            nc.vector.tensor_reduce(out=self.ZT[bt][:, q:q + 1], in_=scr,
                                    op=ALU.add, axis=mybir.AxisListType.X)
            if q < N_QUBITS - 1:
                nc.vector.tensor_tensor(lo, lo, hi, ALU.add)
            w = h

    # ---- top level -------------------------------------------------------

    def _emit_circuit(self):
        for bt in range(NBT):
            self._load_prefix(bt)
        for k in range(KSTEPS):
            for bt in range(NBT):
                self._kron_step(k, bt)
        for g in range(N_GATES):
            q = g % 12
            for bt in range(NBT):
                self._fused_gate(g, q, bt)
        Dc = self._build_diags(lambda ci: self.cco(ci))
        for bt in range(NBT):
            self._final_crot(bt, Dc)
        for bt in range(NBT):
            self._observables(bt)


_PROG_CACHE = None


def _get_prog():
    global _PROG_CACHE
    if _PROG_CACHE is None:
        _PROG_CACHE = _Prog()
    return _PROG_CACHE


def _run(inputs, trace=False):
    x = np.asarray(inputs["x"], np.float32)
    fco, kco, pref, cco = _host_payload(
        x, inputs["q_params_rot"], inputs["q_params_enta"])
    # fco: [G,2,B,12] -> per-core tile [128, ((g*2+b)*12+ci)*4+bt]
    cco_tile = np.broadcast_to(cco.reshape(1, NCO), (128, NCO)).copy()
    ident = np.eye(128, dtype=np.float32)
    in_maps = []
    for core in range(N_CORES):
        lo = core * B_CORE
        f = fco[:, :, lo:lo + B_CORE, :]                  # [G,2,512,12]
        f = f.reshape(N_GATES, 2, NBT, 128, NCO)
        f = np.ascontiguousarray(np.transpose(f, (3, 0, 1, 4, 2)))
        k = kco[:, lo:lo + B_CORE, :].reshape(KSTEPS, NBT, 128, NCO)
        k = np.ascontiguousarray(np.transpose(k, (2, 0, 3, 1)))
        p = pref[lo:lo + B_CORE].reshape(NBT, 128, 64)    # complex
        pr = np.empty((128, NBT, 2, 64), np.float32)
        pr[:, :, 0, :] = np.moveaxis(p.real, 1, 0)
        pr[:, :, 1, :] = np.moveaxis(p.imag, 1, 0)
        in_maps.append({
            "fco": f.reshape(128, FCO_W),
            "kco": k.reshape(128, KCO_W),
            "pref": np.ascontiguousarray(pr.reshape(128, PREF_W)),
            "cco": cco_tile,
            "ident": ident,
        })
    prog = _get_prog()
    res = run_bass_kernel_spmd(prog.nc, in_maps, list(range(N_CORES)),
                               trace=trace)
    z = np.concatenate([res.results[c]["z"] for c in range(N_CORES)], axis=0)
    return z.astype(np.float32), res


def kernel(**inputs):
    z, _ = _run(inputs, trace=False)
    return z


# revision 17
# speedup vs baseline: 1.4380x; 1.0237x over previous
"""Trainium2 Bass kernel for a 12-qubit batched PennyLane-style circuit.

Fused formulation (validated in mirror.py):
  Circuit = prod_l [C_l P_l], P_l = tensor of per-sample 1q gates G_{l,q},
  C_l = ring of fixed CRots CR_{l,c} (ctrl c, target c+1 mod 12).
  Rewrite: C_l P_l = CR_{l,11} . [CR_{l,10} G_{l,11}] ... [CR_{l,0} G_{l,1}] . G_{l,0}
  so each CRot fuses with the per-sample gate on its target wire into a
  2-qubit gate that costs the same as a 1q gate (per-sample coefficients on
  both ctrl branches). The wrap CRot CR_{l,11} fuses into layer l+1's G on
  wire 0. Layer 0 (applied to |0..0>) becomes an MPS ladder product state:
  host builds the 64-amplitude prefix over wires 0-5, the device doubles it
  6 times (wires 6-11) with the ladder CRots folded in. Only CR_{3,11}
  remains as a real gate (emitted as a ping-pong gate with identity copies
  on the ctrl=0 half).

Distribution: pure data parallel, 4096 samples -> 8 cores x 512; per core
4 batch tiles of 128 samples (partitions). State = fp32 re/im planes,
batch on partitions, 5 ping-pong buffers (4 bt + 1 spare) so every gate
writes a fresh buffer: no copybacks, chains accumulate in the destination.

Engine split per gate site (gate, bt): ctrl=1 branch -> TensorE as diagonal
matmuls (per-sample coeffs on the diag) accumulating 4 terms in PSUM;
ctrl=0 branch -> 3 chains on DVE + 1 on GpSimd (4-term mult-add chains,
chain start on ScalarE); PSUM evictions + diag builds on ScalarE.
"""

import numpy as np

import concourse.bass as bass
import concourse.bacc as bacc
import concourse.mybir as mybir
from concourse.tile import TileContext
from concourse.bass_utils import run_bass_kernel_spmd

F32 = mybir.dt.float32
F32R = mybir.dt.float32r
F16 = mybir.dt.float16
STATE_DT = F16
ALU = mybir.AluOpType
AF = mybir.ActivationFunctionType

N_QUBITS = 12
N_LAYERS = 4
DIM = 4096
B_FULL = 4096
N_CORES = 8
B_CORE = B_FULL // N_CORES   # 512
NBT = B_CORE // 128          # 4

# coefficient plane order for a 2x2 complex gate [[a,b],[c,d]]
(ARE, AIM, MAIM, BRE, BIM, MBIM,
 CRE, CIM, MCIM, DRE, DIM_, MDIM) = range(12)
NCO = 12

N_GATES = 36                     # layers 1-3, 12 fused gates each
FCO_W = N_GATES * 2 * NCO * NBT  # 3456
KSTEPS = 6                       # device kron steps: wires 6..11
KCO_W = KSTEPS * NCO * NBT       # 288
PREF_W = NBT * 2 * 64            # 512

# per-output chain term tables: (t_out, comp_out) -> 4x (plane, t_in, comp_in)
TERMS = {
    (0, 0): ((ARE, 0, 0), (MAIM, 0, 1), (BRE, 1, 0), (MBIM, 1, 1)),
    (0, 1): ((AIM, 0, 0), (ARE, 0, 1), (BIM, 1, 0), (BRE, 1, 1)),
    (1, 0): ((CRE, 0, 0), (MCIM, 0, 1), (DRE, 1, 0), (MDIM, 1, 1)),
    (1, 1): ((CIM, 0, 0), (CRE, 0, 1), (DIM_, 1, 0), (DRE, 1, 1)),
}
OUTS = ((0, 0), (0, 1), (1, 0), (1, 1))

# ---------------------------------------------------------------------------
# engine plan knobs
# 16 chunk-slots per site: [b=1 outputs o0h0,o0h1,o1h0,...,o3h1, then b=0 ...]
# gpsimd can NOT run scalar_tensor_tensor (neuronxcc rejects it on Pool), so
# chains are DVE-only; gpsimd builds diags / copies / adds instead.
SITE_PLAN = ("pe",) * 8 + ("pe",) + ("dve",) * 7
EVICT_ROT = ("act",)              # psum eviction engines
DIAG_ROT = ("act",)               # diag build engines
CROT_PLAN = ("pe", "dve", "pe", "dve")      # final CRot ctrl=1 outputs
CROT_COPY_ROT = ("act", "dve", "act", "dve")  # final CRot ctrl=0 copies

# ---------------------------------------------------------------------------
# host-side gate algebra
# ---------------------------------------------------------------------------


def _rz(t):
    e = np.exp(-0.5j * t)
    z = np.zeros_like(e)
    return np.stack([np.stack([e, z], -1), np.stack([z, np.conj(e)], -1)], -2)


def _ry(t):
    c = np.cos(t / 2).astype(np.complex128)
    s = np.sin(t / 2).astype(np.complex128)
    return np.stack([np.stack([c, -s], -1), np.stack([s, c], -1)], -2)


def _rx(t):
    c = np.cos(t / 2).astype(np.complex128)
    s = np.sin(t / 2).astype(np.complex128)
    return np.stack([np.stack([c, -1j * s], -1), np.stack([-1j * s, c], -1)], -2)


def _rot(phi, theta, omega):
    # PennyLane Rot = RZ(omega) @ RY(theta) @ RZ(phi)
    return _rz(omega) @ _ry(theta) @ _rz(phi)


def _coef_planes(g):
    """g: [..., 2, 2] complex -> [..., 12] float32 coefficient planes."""
    a, b = g[..., 0, 0], g[..., 0, 1]
    c, d = g[..., 1, 0], g[..., 1, 1]
    cols = [a.real, a.imag, -a.imag, b.real, b.imag, -b.imag,
            c.real, c.imag, -c.imag, d.real, d.imag, -d.imag]
    return np.stack(cols, -1).astype(np.float32)


def _host_gates(x, q_params_rot, q_params_enta):
    x = np.asarray(x, np.float64)
    pr = np.asarray(q_params_rot, np.float64)
    pe = np.asarray(q_params_enta, np.float64)
    enc = np.einsum('qbij,qbjk->qbik',
                    _ry(x[:, 3, :].T),
                    np.einsum('qbij,qbjk->qbik', _rz(x[:, 2, :].T),
                              np.einsum('qbij,qbjk->qbik',
                                        _rx(x[:, 1, :].T), _ry(x[:, 0, :].T))))
    rot = _rot(pr[..., 0], pr[..., 1], pr[..., 2])      # [L,Q,2,2]
    G = np.einsum('lqij,qbjk->lqbik', rot, enc)         # [L,Q,B,2,2]
    U = _rot(pe[..., 0], pe[..., 1], pe[..., 2])        # [L,Q,2,2]
    return G, U


def _host_payload(x, q_params_rot, q_params_enta):
    """Full-batch coefficient arrays: fco [B,3456-layout], kco, pref, cco."""
    B = x.shape[0]
    G, U = _host_gates(x, q_params_rot, q_params_enta)

    # fused gate branch matrices
    fco = np.empty((N_GATES, 2, B, NCO), np.float32)
    for g in range(N_GATES):
        l, q = 1 + g // 12, g % 12
        M0 = G[l, q]
        if q == 0:
            M1 = np.einsum('bij,jk->bik', G[l, 0], U[l - 1, 11])
        else:
            M1 = np.einsum('ij,bjk->bik', U[l, q - 1], G[l, q])
        fco[g, 0] = _coef_planes(M0)
        fco[g, 1] = _coef_planes(M1)

    # kron ladder steps q=6..11: chi_q(b)[j] as a 2x2 "matrix" M[b][j]
    kco = np.empty((KSTEPS, B, NCO), np.float32)
    for k in range(KSTEPS):
        q = 6 + k
        v0 = G[0, q, :, :, 0]                            # [B,2]
        v1 = np.einsum('ij,bj->bi', U[0, q - 1], v0)
        KM = np.empty((B, 2, 2), np.complex128)
        KM[:, 0, :] = v0
        KM[:, 1, :] = v1
        kco[k] = _coef_planes(KM)

    # 64-amplitude prefix over wires 0-5 (ladder CRots folded)
    pref = G[0, 0, :, :, 0]                              # [B,2]
    for q in range(1, 6):
        v0 = G[0, q, :, :, 0]
        v1 = np.einsum('ij,bj->bi', U[0, q - 1], v0)
        w = pref.shape[1]
        new = np.empty((B, 2 * w), np.complex128)
        nv = new.reshape(B, w // 2, 2, 2) if w > 1 else None
        if w == 1:
            raise AssertionError
        old = pref.reshape(B, w // 2, 2)
        for b in (0, 1):
            chi = v0 if b == 0 else v1
            for j in (0, 1):
                nv[:, :, b, j] = old[:, :, b] * chi[:, j][:, None]
        pref = new                                       # [B, 64]

    cco = _coef_planes(U[3, 11])                         # [12]
    return fco, kco, pref, cco


# ---------------------------------------------------------------------------
# bass program
# ---------------------------------------------------------------------------


class _Prog:
    def __init__(self):
        nc = bacc.Bacc("TRN2", target_bir_lowering=False, debug=False)
        self.nc = nc
        self.fco_d = nc.declare_dram_parameter("fco", [128, FCO_W], F32,
                                               isOutput=False)
        self.kco_d = nc.declare_dram_parameter("kco", [128, KCO_W], F32,
                                               isOutput=False)
        self.pref_d = nc.declare_dram_parameter("pref", [128, PREF_W], F32,
                                                isOutput=False)
        self.cco_d = nc.declare_dram_parameter("cco", [128, NCO], F32,
                                               isOutput=False)
        self.idn_d = nc.declare_dram_parameter("ident", [128, 128], F32,
                                               isOutput=False)
        self.z_d = nc.declare_dram_parameter("z", [B_CORE, N_QUBITS], F32,
                                             isOutput=True)
        self._ectr = 0      # eviction engine rotation
        self._dctr = 0      # diag engine rotation
        self._cctr = 0      # chain plan rotation
        with TileContext(nc) as tc:
            self.tc = tc
            with tc.tile_pool(name="main", bufs=1) as pool, \
                    tc.tile_pool(name="dpool", bufs=24) as dpool, \
                    tc.tile_pool(name="psum", bufs=8, space="PSUM") as ppool:
                self.dpool = dpool
                self.ppool = ppool
                # 5 ping-pong plane-pair buffers [re | im], each [128, 8192]
                self.BUF = [pool.tile([128, 2 * DIM], STATE_DT, name=f"st{i}",
                                      tag=f"st{i}") for i in range(5)]
                self.FC = pool.tile([128, FCO_W], F32, tag="fc")
                self.KC = pool.tile([128, KCO_W], F32, tag="kc")
                self.CC = pool.tile([128, NCO], F32, tag="cc")
                self.PS = pool.tile([128, PREF_W], F32, tag="prefs")
                self.I128 = pool.tile([128, 128], F32, tag="ident")
                self.ZT = [pool.tile([128, 16], F32, name=f"z{bt}",
                                     tag=f"z{bt}") for bt in range(NBT)]
                self.SCR = [pool.tile([128, 2048], F32, name=f"scr{i}",
                                      tag=f"scr{i}") for i in range(2)]
                self.cur = [0, 1, 2, 3]
                self.spare = 4

                nc.sync.dma_start(out=self.FC[:], in_=self.fco_d[:])
                nc.sync.dma_start(out=self.KC[:], in_=self.kco_d[:])
                nc.sync.dma_start(out=self.CC[:], in_=self.cco_d[:])
                nc.sync.dma_start(out=self.PS[:], in_=self.pref_d[:])
                nc.sync.dma_start(out=self.I128[:], in_=self.idn_d[:])

                self._emit_circuit()

                for bt in range(NBT):
                    nc.sync.dma_start(
                        out=self.z_d[bt * 128:(bt + 1) * 128, :],
                        in_=self.ZT[bt][:, 0:N_QUBITS])
        nc.compile()

    # ---- AP helpers -----------------------------------------------------

    def plane(self, buf, comp):
        return self.BUF[buf][:, comp * DIM:(comp + 1) * DIM]

    def fsl(self, buf, comp, q, b, t):
        """F-gate slice (wires q-1,q), q in 1..11: ctrl bit=b, target bit=t."""
        p = self.plane(buf, comp)
        if q == 11:
            v = p.rearrange("p (a c t) -> p a c t", c=2, t=2)
            return v[:, :, b, t]                     # [p, 1024] stride 4
        A = 1 << (q - 1)
        R = 1 << (11 - q)
        v = p.rearrange("p (a c t r) -> p a c t r", a=A, c=2, t=2, r=R)
        return v[:, :, b, t, :]                      # [p, A, R]

    def wsl(self, buf, comp, b, t):
        """Wrap-gate slice (wires 11,0): ctrl a11 (LSB)=b, target a0 (MSB)=t."""
        p = self.plane(buf, comp)
        v = p.rearrange("p (t a c) -> p t a c", t=2, c=2)
        return v[:, t, :, b]                         # [p, 1024] stride 2

    def fco(self, g, b, ci, bt):
        idx = ((g * 2 + b) * NCO + ci) * NBT + bt
        return self.FC[:, idx:idx + 1]

    def kco(self, k, ci, bt):
        idx = (k * NCO + ci) * NBT + bt
        return self.KC[:, idx:idx + 1]

    def cco(self, ci):
        return self.CC[:, ci:ci + 1]

    @staticmethod
    def _chunk(view, idx, csz):
        """csz-wide column chunk of an AP shaped [128, w] or [128, n, s]."""
        shp = view.shape[1:]
        if len(shp) == 1:
            return view[:, idx * csz:(idx + 1) * csz]
        n, s = shp
        if s >= csz:
            m = s // csz
            return view[:, idx // m, (idx % m) * csz:(idx % m + 1) * csz]
        na = csz // s
        return view[:, idx * na:(idx + 1) * na, :]

    def _eng(self, name):
        return {"dve": self.nc.vector, "gps": self.nc.gpsimd}[name]

    # ---- gate emission --------------------------------------------------

    def _build_diags(self, co):
        nc = self.nc
        D = {}
        for ci in range(NCO):
            d = self.dpool.tile([128, 128], STATE_DT, name="dg", tag="dg")
            e = DIAG_ROT[self._dctr % len(DIAG_ROT)]
            self._dctr += 1
            if e == "act":
                nc.scalar.activation(d[:], self.I128[:], AF.Copy,
                                     scale=co(ci))
            else:
                self._eng(e).tensor_scalar(d[:], self.I128[:], co(ci),
                                           None, ALU.mult)
            D[ci] = d
        return D

    def _pe_out(self, dst, srcs, planes, D, nchunks, csz=512):
        """One output slice via TensorE diag matmuls, chunked into PSUM."""
        nc = self.nc
        for h in range(nchunks):
            ps = self.ppool.tile([128, csz], F32, name="ps", tag="ps")
            for k in range(4):
                nc.tensor.matmul(out=ps[:], lhsT=D[planes[k]][:],
                                 rhs=self._chunk(srcs[k], h, csz),
                                 start=(k == 0), stop=(k == 3))
            dc = self._chunk(dst, h, csz)
            src = ps[:]
            if len(dc.shape) > 2:
                src = src.rearrange("p (a r) -> p a r", r=dc.shape[-1])
            e = EVICT_ROT[self._ectr % len(EVICT_ROT)]
            self._ectr += 1
            if e == "act":
                nc.scalar.copy(dc, src)
            else:
                self._eng(e).tensor_copy(out=dc, in_=src)

    def _emit_site(self, sl_src, sl_dst, co):
        """Emit one fused-gate site: slices are dicts (b,t,comp)->AP.
        Work units follow SITE_PLAN (16 chunk slots); adjacent same-engine
        chunks of one output are coalesced into full-width ops."""
        nc = self.nc
        pe_units = []   # (branch, dst-view, src-views[4], planes, nchunks)
        chains = []     # (engine, dst, srcs[4], coefs[4])
        pe_branches = set()
        for bi, b in enumerate((1, 0)):
            for oi, (t, c) in enumerate(OUTS):
                dst = sl_dst[(b, t, c)]
                terms = TERMS[(t, c)]
                srcs = [sl_src[(b, ti, ci)] for (_, ti, ci) in terms]
                planes = [pl for (pl, _, _) in terms]
                e0 = SITE_PLAN[bi * 8 + oi * 2]
                e1 = SITE_PLAN[bi * 8 + oi * 2 + 1]
                if e0 == e1:
                    if e0 == "pe":
                        pe_branches.add(b)
                        pe_units.append((b, dst, srcs, planes, 2))
                    else:
                        chains.append((e0, dst, srcs,
                                       [co(b, pl) for pl in planes]))
                else:
                    for h, e in ((0, e0), (1, e1)):
                        dc = self._chunk2(dst, h)
                        sc = [self._chunk2(s, h) for s in srcs]
                        if e == "pe":
                            pe_branches.add(b)
                            pe_units.append((b, dc, sc, planes, 1))
                        else:
                            chains.append((e, dc, sc,
                                           [co(b, pl) for pl in planes]))
        # emission order: chain starts (ScalarE) first so DVE can begin,
        # then diags + matmuls, then STT rounds
        for (e, dst, srcs, coefs) in chains:
            nc.scalar.activation(dst, srcs[0], AF.Copy, scale=coefs[0])
        D = {b: self._build_diags(lambda ci: co(b, ci)) for b in pe_branches}
        for (b, dst, srcs, planes, nch) in pe_units:
            self._pe_out(dst, srcs, planes, D[b], nchunks=nch)
        for k in range(1, 4):
            for (e, dst, srcs, coefs) in chains:
                self._eng(e).scalar_tensor_tensor(dst, srcs[k], coefs[k],
                                                  dst, ALU.mult, ALU.add)

    def _chunk2(self, view, h):
        """half-split a 1024-col slice into 512-col chunks"""
        return self._chunk(view, h, 512)

    def _emit_chains(self, chains):
        """chains: list of (eng, dst, srcs[4], coefs[4]); starts on ScalarE,
        then STT accumulation interleaved round-robin per engine."""
        nc = self.nc
        for (e, dst, srcs, coefs) in chains:
            nc.scalar.activation(dst, srcs[0], AF.Copy, scale=coefs[0])
        for k in range(1, 4):
            for (e, dst, srcs, coefs) in chains:
                self._eng(e).scalar_tensor_tensor(dst, srcs[k], coefs[k],
                                                  dst, ALU.mult, ALU.add)

    def _fused_gate(self, g, q, bt):
        src, dst = self.cur[bt], self.spare
        sl_src, sl_dst = {}, {}
        for b in (0, 1):
            for t in (0, 1):
                for c in (0, 1):
                    if q == 0:
                        sl_src[(b, t, c)] = self.wsl(src, c, b, t)
                        sl_dst[(b, t, c)] = self.wsl(dst, c, b, t)
                    else:
                        sl_src[(b, t, c)] = self.fsl(src, c, q, b, t)
                        sl_dst[(b, t, c)] = self.fsl(dst, c, q, b, t)
        co = lambda b, ci: self.fco(g, b, ci, bt)
        self._emit_site(sl_src, sl_dst, co)
        self.spare, self.cur[bt] = self.cur[bt], self.spare

    # ---- layer 0: prefix load + kron ladder ------------------------------

    def _load_prefix(self, bt):
        nc = self.nc
        dst = self.BUF[self.cur[bt]].rearrange("p (c n) -> p c n", c=2)
        src = self.PS[:, bt * 128:(bt + 1) * 128].rearrange(
            "p (c n) -> p c n", c=2)
        nc.scalar.copy(dst[:, :, 0:64], src)

    def _kron_step(self, k, bt):
        """Double width w -> 2w appending wire q=6+k, ladder CRot folded."""
        nc = self.nc
        w = 64 << k
        src, dst = self.cur[bt], self.spare
        units = []
        for b in (0, 1):
            for j in (0, 1):
                pl = 3 * (2 * b + j)     # re plane of entry [b][j]
                for comp in (0, 1):
                    old_re = self.plane(src, 0)[:, 0:w].rearrange(
                        "p (a pb) -> p a pb", pb=2)[:, :, b]
                    old_im = self.plane(src, 1)[:, 0:w].rearrange(
                        "p (a pb) -> p a pb", pb=2)[:, :, b]
                    d = self.plane(dst, comp)[:, 0:2 * w].rearrange(
                        "p (a pb j) -> p a pb j", pb=2, j=2)[:, :, b, j]
                    if comp == 0:
                        # re = old_re*chi_re + old_im*(-chi_im)
                        units.append((d, old_re, self.kco(k, pl, bt),
                                      old_im, self.kco(k, pl + 2, bt)))
                    else:
                        # im = old_re*chi_im + old_im*chi_re
                        units.append((d, old_re, self.kco(k, pl + 1, bt),
                                      old_im, self.kco(k, pl, bt)))
        for (d, s0, c0, s1, c1) in units:
            nc.scalar.activation(d, s0, AF.Copy, scale=c0)
        for (d, s0, c0, s1, c1) in units:
            nc.vector.scalar_tensor_tensor(d, s1, c1, d, ALU.mult, ALU.add)
        self.spare, self.cur[bt] = self.cur[bt], self.spare

    # ---- final CRot ------------------------------------------------------

    def _final_crot(self, bt, D):
        """CR_{3,11}: ctrl=1 gets U, ctrl=0 identity copies; ping-pong."""
        nc = self.nc
        src, dst = self.cur[bt], self.spare
        # ctrl=0: plain copies
        for i, (t, c) in enumerate(OUTS):
            s = self.wsl(src, c, 0, t)
            d = self.wsl(dst, c, 0, t)
            e = CROT_COPY_ROT[i % len(CROT_COPY_ROT)]
            if e == "act":
                nc.scalar.copy(d, s)
            elif e == "dve":
                nc.vector.tensor_copy(out=d, in_=s)
            else:
                nc.gpsimd.tensor_copy(out=d, in_=s)
        # ctrl=1: gate with fixed broadcast coeffs
        chains = []
        for oi, (t, c) in enumerate(OUTS):
            dst_ap = self.wsl(dst, c, 1, t)
            terms = TERMS[(t, c)]
            srcs = [self.wsl(src, ci, 1, ti) for (_, ti, ci) in terms]
            planes = [pl for (pl, _, _) in terms]
            e = CROT_PLAN[oi % len(CROT_PLAN)]
            if e == "pe":
                self._pe_out(dst_ap, srcs, planes, D, nchunks=2)
            else:
                chains.append((e, dst_ap, srcs,
                               [self.cco(pl) for pl in planes]))
        self._emit_chains(chains)
        self.spare, self.cur[bt] = self.cur[bt], self.spare

    # ---- observables -----------------------------------------------------

    def _observables(self, bt):
        """probs overwrite the re plane in place; im plane is scratch."""
        nc = self.nc
        buf = self.cur[bt]
        re = self.plane(buf, 0)
        im = self.plane(buf, 1)
        for h in range(4):
            sl = slice(h * 1024, (h + 1) * 1024)
            nc.scalar.activation(re[:, sl], re[:, sl], AF.Square)
            nc.scalar.activation(im[:, sl], im[:, sl], AF.Square)
            nc.vector.tensor_tensor(re[:, sl], re[:, sl], im[:, sl], ALU.add)
        w = DIM
        for q in range(N_QUBITS):
            h = w // 2
            lo, hi = re[:, 0:h], re[:, h:w]
            # (lo - hi) into f32 scratch, then reduce into ZT
            scr = self.SCR[bt % 2][:, 0:h]
            nc.vector.tensor_tensor(scr, lo, hi, ALU.subtract)
            nc.vector.tensor_reduce(out=self.ZT[bt][:, q:q + 1], in_=scr,
                                    op=ALU.add, axis=mybir.AxisListType.X)
            if q < N_QUBITS - 1:
                nc.vector.tensor_tensor(lo, lo, hi, ALU.add)
            w = h

    # ---- top level -------------------------------------------------------

    def _emit_circuit(self):
        for bt in range(NBT):
            self._load_prefix(bt)
        for k in range(KSTEPS):
            for bt in range(NBT):
                self._kron_step(k, bt)
        for g in range(N_GATES):
            q = g % 12
            for bt in range(NBT):
                self._fused_gate(g, q, bt)
        Dc = self._build_diags(lambda ci: self.cco(ci))
        for bt in range(NBT):
            self._final_crot(bt, Dc)
        for bt in range(NBT):
            self._observables(bt)


_PROG_CACHE = None


def _get_prog():
    global _PROG_CACHE
    if _PROG_CACHE is None:
        _PROG_CACHE = _Prog()
    return _PROG_CACHE


def _run(inputs, trace=False):
    x = np.asarray(inputs["x"], np.float32)
    fco, kco, pref, cco = _host_payload(
        x, inputs["q_params_rot"], inputs["q_params_enta"])
    # fco: [G,2,B,12] -> per-core tile [128, ((g*2+b)*12+ci)*4+bt]
    cco_tile = np.broadcast_to(cco.reshape(1, NCO), (128, NCO)).copy()
    ident = np.eye(128, dtype=np.float32)
    in_maps = []
    for core in range(N_CORES):
        lo = core * B_CORE
        f = fco[:, :, lo:lo + B_CORE, :]                  # [G,2,512,12]
        f = f.reshape(N_GATES, 2, NBT, 128, NCO)
        f = np.ascontiguousarray(np.transpose(f, (3, 0, 1, 4, 2)))
        k = kco[:, lo:lo + B_CORE, :].reshape(KSTEPS, NBT, 128, NCO)
        k = np.ascontiguousarray(np.transpose(k, (2, 0, 3, 1)))
        p = pref[lo:lo + B_CORE].reshape(NBT, 128, 64)    # complex
        pr = np.empty((128, NBT, 2, 64), np.float32)
        pr[:, :, 0, :] = np.moveaxis(p.real, 1, 0)
        pr[:, :, 1, :] = np.moveaxis(p.imag, 1, 0)
        in_maps.append({
            "fco": f.reshape(128, FCO_W),
            "kco": k.reshape(128, KCO_W),
            "pref": np.ascontiguousarray(pr.reshape(128, PREF_W)),
            "cco": cco_tile,
            "ident": ident,
        })
    prog = _get_prog()
    res = run_bass_kernel_spmd(prog.nc, in_maps, list(range(N_CORES)),
                               trace=trace)
    z = np.concatenate([res.results[c]["z"] for c in range(N_CORES)], axis=0)
    return z.astype(np.float32), res


def kernel(**inputs):
    z, _ = _run(inputs, trace=False)
    return z


# revision 25
# speedup vs baseline: 2.2163x; 1.5413x over previous
"""Trainium2 Bass kernel for a 12-qubit batched PennyLane-style circuit.

Fused formulation (validated in mirror.py):
  Circuit = prod_l [C_l P_l], P_l = tensor of per-sample 1q gates G_{l,q},
  C_l = ring of fixed CRots CR_{l,c} (ctrl c, target c+1 mod 12).
  Rewrite: C_l P_l = CR_{l,11} . [CR_{l,10} G_{l,11}] ... [CR_{l,0} G_{l,1}] . G_{l,0}
  so each CRot fuses with the per-sample gate on its target wire into a
  2-qubit gate that costs the same as a 1q gate (per-sample coefficients on
  both ctrl branches). The wrap CRot CR_{l,11} fuses into layer l+1's G on
  wire 0. Layer 0 (applied to |0..0>) becomes an MPS ladder product state:
  host builds the 64-amplitude prefix over wires 0-5, the device doubles it
  6 times (wires 6-11) with the ladder CRots folded in. Only CR_{3,11}
  remains as a real gate (emitted as a ping-pong gate with identity copies
  on the ctrl=0 half).

Distribution: pure data parallel, 4096 samples -> 8 cores x 512; per core
4 batch tiles of 128 samples (partitions). State = fp32 re/im planes,
batch on partitions, 5 ping-pong buffers (4 bt + 1 spare) so every gate
writes a fresh buffer: no copybacks, chains accumulate in the destination.

Engine split per gate site (gate, bt): ctrl=1 branch -> TensorE as diagonal
matmuls (per-sample coeffs on the diag) accumulating 4 terms in PSUM;
ctrl=0 branch -> 3 chains on DVE + 1 on GpSimd (4-term mult-add chains,
chain start on ScalarE); PSUM evictions + diag builds on ScalarE.
"""

import numpy as np

import concourse.bass as bass
import concourse.bacc as bacc
import concourse.mybir as mybir
from concourse.tile import TileContext
from concourse.bass_utils import run_bass_kernel_spmd

F32 = mybir.dt.float32
F32R = mybir.dt.float32r
F16 = mybir.dt.float16
STATE_DT = F16
ALU = mybir.AluOpType
AF = mybir.ActivationFunctionType

N_QUBITS = 12
N_LAYERS = 4
DIM = 4096
B_FULL = 4096
N_CORES = 8
B_CORE = B_FULL // N_CORES   # 512
NBT = B_CORE // 128          # 4

# coefficient plane order for a 2x2 complex gate [[a,b],[c,d]]
(ARE, AIM, MAIM, BRE, BIM, MBIM,
 CRE, CIM, MCIM, DRE, DIM_, MDIM) = range(12)
NCO = 12

N_GATES = 36                     # layers 1-3, 12 fused gates each
FCO_W = N_GATES * 2 * NCO * NBT  # 3456
KSTEPS = 6                       # device kron steps: wires 6..11
KCO_W = KSTEPS * NCO * NBT       # 288
PREF_W = NBT * 2 * 64            # 512

# per-output chain term tables: (t_out, comp_out) -> 4x (plane, t_in, comp_in)
TERMS = {
    (0, 0): ((ARE, 0, 0), (MAIM, 0, 1), (BRE, 1, 0), (MBIM, 1, 1)),
    (0, 1): ((AIM, 0, 0), (ARE, 0, 1), (BIM, 1, 0), (BRE, 1, 1)),
    (1, 0): ((CRE, 0, 0), (MCIM, 0, 1), (DRE, 1, 0), (MDIM, 1, 1)),
    (1, 1): ((CIM, 0, 0), (CRE, 0, 1), (DIM_, 1, 0), (DRE, 1, 1)),
}
OUTS = ((0, 0), (0, 1), (1, 0), (1, 1))

# ---------------------------------------------------------------------------
# engine plan knobs
# gpsimd can NOT run scalar_tensor_tensor (neuronxcc rejects it on Pool) and
# has ~us-scale per-op launch overhead on HW, so chains are DVE-only and
# gpsimd is unused. ScalarE has ~270ns fixed overhead per op, so diag
# matrices are prebuilt on the host and streamed in via (otherwise idle) DMA.
# branch -> full-width outputs on the PE diag-matmul path vs DVE chains
PE_OUTS = {1: ((0, 0), (0, 1), (1, 0), (1, 1)), 0: ((0, 0),)}
CHAIN_OUTS = {0: ((0, 1), (1, 0), (1, 1))}
# staged diag planes per site, in storage order
STAGE_PLANES = tuple((1, ci) for ci in range(NCO)) + \
    tuple((0, ci) for ci in (ARE, MAIM, BRE, MBIM))
NSTAGE = len(STAGE_PLANES)        # 16
N_SITES = N_GATES * NBT           # 144
DIAG_W = N_SITES * NSTAGE * 128   # staged diag dram cols (fp16)
EVICT_ROT = ("act",)              # psum eviction engines
DIAG_ROT = ("act",)               # diag build engines (final CRot only)
CROT_PLAN = ("pe", "dve", "pe", "dve")      # final CRot ctrl=1 outputs
CROT_COPY_ROT = ("act", "dve", "act", "dve")  # final CRot ctrl=0 copies

# ---------------------------------------------------------------------------
# host-side gate algebra
# ---------------------------------------------------------------------------


def _rz(t):
    e = np.exp(-0.5j * t)
    z = np.zeros_like(e)
    return np.stack([np.stack([e, z], -1), np.stack([z, np.conj(e)], -1)], -2)


def _ry(t):
    c = np.cos(t / 2).astype(np.complex128)
    s = np.sin(t / 2).astype(np.complex128)
    return np.stack([np.stack([c, -s], -1), np.stack([s, c], -1)], -2)


def _rx(t):
    c = np.cos(t / 2).astype(np.complex128)
    s = np.sin(t / 2).astype(np.complex128)
    return np.stack([np.stack([c, -1j * s], -1), np.stack([-1j * s, c], -1)], -2)


def _rot(phi, theta, omega):
    # PennyLane Rot = RZ(omega) @ RY(theta) @ RZ(phi)
    return _rz(omega) @ _ry(theta) @ _rz(phi)


def _coef_planes(g):
    """g: [..., 2, 2] complex -> [..., 12] float32 coefficient planes."""
    a, b = g[..., 0, 0], g[..., 0, 1]
    c, d = g[..., 1, 0], g[..., 1, 1]
    cols = [a.real, a.imag, -a.imag, b.real, b.imag, -b.imag,
            c.real, c.imag, -c.imag, d.real, d.imag, -d.imag]
    return np.stack(cols, -1).astype(np.float32)


def _host_gates(x, q_params_rot, q_params_enta):
    x = np.asarray(x, np.float64)
    pr = np.asarray(q_params_rot, np.float64)
    pe = np.asarray(q_params_enta, np.float64)
    enc = np.einsum('qbij,qbjk->qbik',
                    _ry(x[:, 3, :].T),
                    np.einsum('qbij,qbjk->qbik', _rz(x[:, 2, :].T),
                              np.einsum('qbij,qbjk->qbik',
                                        _rx(x[:, 1, :].T), _ry(x[:, 0, :].T))))
    rot = _rot(pr[..., 0], pr[..., 1], pr[..., 2])      # [L,Q,2,2]
    G = np.einsum('lqij,qbjk->lqbik', rot, enc)         # [L,Q,B,2,2]
    U = _rot(pe[..., 0], pe[..., 1], pe[..., 2])        # [L,Q,2,2]
    return G, U


def _host_payload(x, q_params_rot, q_params_enta):
    """Full-batch coefficient arrays: fco [B,3456-layout], kco, pref, cco."""
    B = x.shape[0]
    G, U = _host_gates(x, q_params_rot, q_params_enta)

    # fused gate branch matrices
    fco = np.empty((N_GATES, 2, B, NCO), np.float32)
    for g in range(N_GATES):
        l, q = 1 + g // 12, g % 12
        M0 = G[l, q]
        if q == 0:
            M1 = np.einsum('bij,jk->bik', G[l, 0], U[l - 1, 11])
        else:
            M1 = np.einsum('ij,bjk->bik', U[l, q - 1], G[l, q])
        fco[g, 0] = _coef_planes(M0)
        fco[g, 1] = _coef_planes(M1)

    # kron ladder steps q=6..11: chi_q(b)[j] as a 2x2 "matrix" M[b][j]
    kco = np.empty((KSTEPS, B, NCO), np.float32)
    for k in range(KSTEPS):
        q = 6 + k
        v0 = G[0, q, :, :, 0]                            # [B,2]
        v1 = np.einsum('ij,bj->bi', U[0, q - 1], v0)
        KM = np.empty((B, 2, 2), np.complex128)
        KM[:, 0, :] = v0
        KM[:, 1, :] = v1
        kco[k] = _coef_planes(KM)

    # 64-amplitude prefix over wires 0-5 (ladder CRots folded)
    pref = G[0, 0, :, :, 0]                              # [B,2]
    for q in range(1, 6):
        v0 = G[0, q, :, :, 0]
        v1 = np.einsum('ij,bj->bi', U[0, q - 1], v0)
        w = pref.shape[1]
        new = np.empty((B, 2 * w), np.complex128)
        nv = new.reshape(B, w // 2, 2, 2) if w > 1 else None
        if w == 1:
            raise AssertionError
        old = pref.reshape(B, w // 2, 2)
        for b in (0, 1):
            chi = v0 if b == 0 else v1
            for j in (0, 1):
                nv[:, :, b, j] = old[:, :, b] * chi[:, j][:, None]
        pref = new                                       # [B, 64]

    cco = _coef_planes(U[3, 11])                         # [12]
    return fco, kco, pref, cco


def _host_diags(fco, lo):
    """Staged diag matrices for one core: [128, N_SITES*NSTAGE*128] fp16.
    Site order: gate-major, bt-minor. Diag k of site (g,bt) holds
    coef-plane STAGE_PLANES[k] of that gate/branch on its diagonal."""
    d = np.zeros((128, N_SITES, NSTAGE, 128), np.float16)
    idx = np.arange(128)
    for g in range(N_GATES):
        for bt in range(NBT):
            s = g * NBT + bt
            sl = slice(lo + bt * 128, lo + (bt + 1) * 128)
            for k, (b, ci) in enumerate(STAGE_PLANES):
                d[idx, s, k, idx] = fco[g, b, sl, ci].astype(np.float16)
    return d.reshape(128, DIAG_W)


# ---------------------------------------------------------------------------
# bass program
# ---------------------------------------------------------------------------


class _Prog:
    def __init__(self):
        nc = bacc.Bacc("TRN2", target_bir_lowering=False, debug=False)
        self.nc = nc
        self.fco_d = nc.declare_dram_parameter("fco", [128, FCO_W], F32,
                                               isOutput=False)
        self.diag_d = nc.declare_dram_parameter("diag", [128, DIAG_W], F16,
                                                isOutput=False)
        self.kco_d = nc.declare_dram_parameter("kco", [128, KCO_W], F32,
                                               isOutput=False)
        self.pref_d = nc.declare_dram_parameter("pref", [128, PREF_W], F32,
                                                isOutput=False)
        self.cco_d = nc.declare_dram_parameter("cco", [128, NCO], F32,
                                               isOutput=False)
        self.idn_d = nc.declare_dram_parameter("ident", [128, 128], F32,
                                               isOutput=False)
        self.z_d = nc.declare_dram_parameter("z", [B_CORE, N_QUBITS], F32,
                                             isOutput=True)
        self._ectr = 0      # eviction engine rotation
        self._dctr = 0      # diag engine rotation
        self._cctr = 0      # chain plan rotation
        with TileContext(nc) as tc:
            self.tc = tc
            with tc.tile_pool(name="main", bufs=1) as pool, \
                    tc.tile_pool(name="dpool", bufs=14) as dpool, \
                    tc.tile_pool(name="dstage", bufs=6) as dstage, \
                    tc.tile_pool(name="psum", bufs=4, space="PSUM") as ppool:
                self.dpool = dpool
                self.dstage = dstage
                self.ppool = ppool
                # 5 ping-pong plane-pair buffers [re | im], each [128, 8192]
                self.BUF = [pool.tile([128, 2 * DIM], STATE_DT, name=f"st{i}",
                                      tag=f"st{i}") for i in range(5)]
                self.FC = pool.tile([128, FCO_W], F32, tag="fc")
                self.KC = pool.tile([128, KCO_W], F32, tag="kc")
                self.CC = pool.tile([128, NCO], F32, tag="cc")
                self.PS = pool.tile([128, PREF_W], F32, tag="prefs")
                self.I128 = pool.tile([128, 128], F32, tag="ident")
                self.ZT = [pool.tile([128, 16], F32, name=f"z{bt}",
                                     tag=f"z{bt}") for bt in range(NBT)]
                self.SCR = [pool.tile([128, 2048], F32, name=f"scr{i}",
                                      tag=f"scr{i}") for i in range(2)]
                self.cur = [0, 1, 2, 3]
                self.spare = 4

                nc.sync.dma_start(out=self.FC[:], in_=self.fco_d[:])
                nc.sync.dma_start(out=self.KC[:], in_=self.kco_d[:])
                nc.sync.dma_start(out=self.CC[:], in_=self.cco_d[:])
                nc.sync.dma_start(out=self.PS[:], in_=self.pref_d[:])
                nc.sync.dma_start(out=self.I128[:], in_=self.idn_d[:])

                self._emit_circuit()

                for bt in range(NBT):
                    nc.sync.dma_start(
                        out=self.z_d[bt * 128:(bt + 1) * 128, :],
                        in_=self.ZT[bt][:, 0:N_QUBITS])
        nc.compile()

    # ---- AP helpers -----------------------------------------------------

    def plane(self, buf, comp):
        return self.BUF[buf][:, comp * DIM:(comp + 1) * DIM]

    def fsl(self, buf, comp, q, b, t):
        """F-gate slice (wires q-1,q), q in 1..11: ctrl bit=b, target bit=t."""
        p = self.plane(buf, comp)
        if q == 11:
            v = p.rearrange("p (a c t) -> p a c t", c=2, t=2)
            return v[:, :, b, t]                     # [p, 1024] stride 4
        A = 1 << (q - 1)
        R = 1 << (11 - q)
        v = p.rearrange("p (a c t r) -> p a c t r", a=A, c=2, t=2, r=R)
        return v[:, :, b, t, :]                      # [p, A, R]

    def wsl(self, buf, comp, b, t):
        """Wrap-gate slice (wires 11,0): ctrl a11 (LSB)=b, target a0 (MSB)=t."""
        p = self.plane(buf, comp)
        v = p.rearrange("p (t a c) -> p t a c", t=2, c=2)
        return v[:, t, :, b]                         # [p, 1024] stride 2

    def fco(self, g, b, ci, bt):
        idx = ((g * 2 + b) * NCO + ci) * NBT + bt
        return self.FC[:, idx:idx + 1]

    def kco(self, k, ci, bt):
        idx = (k * NCO + ci) * NBT + bt
        return self.KC[:, idx:idx + 1]

    def cco(self, ci):
        return self.CC[:, ci:ci + 1]

    @staticmethod
    def _chunk(view, idx, csz):
        """csz-wide column chunk of an AP shaped [128, w] or [128, n, s]."""
        shp = view.shape[1:]
        if len(shp) == 1:
            return view[:, idx * csz:(idx + 1) * csz]
        n, s = shp
        if s >= csz:
            m = s // csz
            return view[:, idx // m, (idx % m) * csz:(idx % m + 1) * csz]
        na = csz // s
        return view[:, idx * na:(idx + 1) * na, :]

    def _eng(self, name):
        return {"dve": self.nc.vector, "gps": self.nc.gpsimd}[name]

    # ---- gate emission --------------------------------------------------

    def _build_diags(self, co):
        nc = self.nc
        D = {}
        for ci in range(NCO):
            d = self.dpool.tile([128, 128], STATE_DT, name="dg", tag="dg")
            e = DIAG_ROT[self._dctr % len(DIAG_ROT)]
            self._dctr += 1
            if e == "act":
                nc.scalar.activation(d[:], self.I128[:], AF.Copy,
                                     scale=co(ci))
            else:
                self._eng(e).tensor_scalar(d[:], self.I128[:], co(ci),
                                           None, ALU.mult)
            D[ci] = d
        return D

    def _pe_out(self, dst, srcs, planes, D):
        """One full-width (1024) output via TensorE diag matmuls into one
        wide PSUM tile (two 512 accumulation regions), one eviction."""
        nc = self.nc
        ps = self.ppool.tile([128, 1024], F32, name="ps", tag="ps")
        for k in range(4):
            for h in range(2):
                nc.tensor.matmul(out=ps[:, h * 512:(h + 1) * 512],
                                 lhsT=D[planes[k]][:],
                                 rhs=self._chunk(srcs[k], h, 512),
                                 start=(k == 0), stop=(k == 3))
        src = ps[:]
        if len(dst.shape) > 2:
            src = src.rearrange("p (a r) -> p a r", r=dst.shape[-1])
        nc.scalar.copy(dst, src)

    def _emit_site(self, site, sl_src, sl_dst, co):
        """Emit one fused-gate site: slices are dicts (b,t,comp)->AP.
        PE_OUTS outputs go to TensorE (host-staged diags, plane-grouped
        matmuls for lhsT reuse, one wide PSUM tile + eviction per output);
        CHAIN_OUTS outputs are 4-term DVE chains with ScalarE starts."""
        nc = self.nc
        chains = []     # (engine, dst, srcs[4], coefs[4])
        for b, outs in CHAIN_OUTS.items():
            for (t, c) in outs:
                terms = TERMS[(t, c)]
                chains.append(("dve", sl_dst[(b, t, c)],
                               [sl_src[(b, ti, ci)] for (_, ti, ci) in terms],
                               [co(b, pl) for (pl, _, _) in terms]))
        # chain starts (ScalarE) first so DVE can begin
        for (e, dst, srcs, coefs) in chains:
            nc.scalar.activation(dst, srcs[0], AF.Copy, scale=coefs[0])
        # staged diags for this site arrive by DMA
        stg = self.dstage.tile([128, NSTAGE * 128], F16, name="stg",
                               tag="stg")
        nc.sync.dma_start(
            out=stg[:],
            in_=self.diag_d[:, site * NSTAGE * 128:(site + 1) * NSTAGE * 128])
        # PE: one [128,1024] psum tile per output; matmuls grouped by plane
        psums, nterm = {}, {}
        for b, outs in PE_OUTS.items():
            for o in outs:
                psums[(b, o)] = self.ppool.tile([128, 1024], F32, name="ps",
                                                tag="ps")
                nterm[(b, o)] = [0, 0]
        for k, (b, ci) in enumerate(STAGE_PLANES):
            lhsT = stg[:, k * 128:(k + 1) * 128]
            for o in PE_OUTS.get(b, ()):
                terms = TERMS[o]
                for (pl, ti, cii) in terms:
                    if pl != ci:
                        continue
                    src = sl_src[(b, ti, cii)]
                    ps = psums[(b, o)]
                    for h in range(2):
                        cnt = nterm[(b, o)]
                        nc.tensor.matmul(
                            out=ps[:, h * 512:(h + 1) * 512],
                            lhsT=lhsT, rhs=self._chunk(src, h, 512),
                            start=(cnt[h] == 0), stop=(cnt[h] == 3))
                        cnt[h] += 1
        for b, outs in PE_OUTS.items():
            for o in outs:
                dst = sl_dst[(b, o[0], o[1])]
                src = psums[(b, o)][:]
                if len(dst.shape) > 2:
                    src = src.rearrange("p (a r) -> p a r", r=dst.shape[-1])
                nc.scalar.copy(dst, src)
        # DVE accumulation rounds
        for k in range(1, 4):
            for (e, dst, srcs, coefs) in chains:
                self._eng(e).scalar_tensor_tensor(dst, srcs[k], coefs[k],
                                                  dst, ALU.mult, ALU.add)

    def _emit_chains(self, chains):
        """chains: list of (eng, dst, srcs[4], coefs[4]); starts on ScalarE,
        then STT accumulation interleaved round-robin per engine."""
        nc = self.nc
        for (e, dst, srcs, coefs) in chains:
            nc.scalar.activation(dst, srcs[0], AF.Copy, scale=coefs[0])
        for k in range(1, 4):
            for (e, dst, srcs, coefs) in chains:
                self._eng(e).scalar_tensor_tensor(dst, srcs[k], coefs[k],
                                                  dst, ALU.mult, ALU.add)

    def _fused_gate(self, g, q, bt):
        src, dst = self.cur[bt], self.spare
        sl_src, sl_dst = {}, {}
        for b in (0, 1):
            for t in (0, 1):
                for c in (0, 1):
                    if q == 0:
                        sl_src[(b, t, c)] = self.wsl(src, c, b, t)
                        sl_dst[(b, t, c)] = self.wsl(dst, c, b, t)
                    else:
                        sl_src[(b, t, c)] = self.fsl(src, c, q, b, t)
                        sl_dst[(b, t, c)] = self.fsl(dst, c, q, b, t)
        co = lambda b, ci: self.fco(g, b, ci, bt)
        self._emit_site(g * NBT + bt, sl_src, sl_dst, co)
        self.spare, self.cur[bt] = self.cur[bt], self.spare

    # ---- layer 0: prefix load + kron ladder ------------------------------

    def _load_prefix(self, bt):
        nc = self.nc
        dst = self.BUF[self.cur[bt]].rearrange("p (c n) -> p c n", c=2)
        src = self.PS[:, bt * 128:(bt + 1) * 128].rearrange(
            "p (c n) -> p c n", c=2)
        nc.scalar.copy(dst[:, :, 0:64], src)

    def _kron_step(self, k, bt):
        """Double width w -> 2w appending wire q=6+k, ladder CRot folded."""
        nc = self.nc
        w = 64 << k
        src, dst = self.cur[bt], self.spare
        units = []
        for b in (0, 1):
            for j in (0, 1):
                pl = 3 * (2 * b + j)     # re plane of entry [b][j]
                for comp in (0, 1):
                    old_re = self.plane(src, 0)[:, 0:w].rearrange(
                        "p (a pb) -> p a pb", pb=2)[:, :, b]
                    old_im = self.plane(src, 1)[:, 0:w].rearrange(
                        "p (a pb) -> p a pb", pb=2)[:, :, b]
                    d = self.plane(dst, comp)[:, 0:2 * w].rearrange(
                        "p (a pb j) -> p a pb j", pb=2, j=2)[:, :, b, j]
                    if comp == 0:
                        # re = old_re*chi_re + old_im*(-chi_im)
                        units.append((d, old_re, self.kco(k, pl, bt),
                                      old_im, self.kco(k, pl + 2, bt)))
                    else:
                        # im = old_re*chi_im + old_im*chi_re
                        units.append((d, old_re, self.kco(k, pl + 1, bt),
                                      old_im, self.kco(k, pl, bt)))
        for (d, s0, c0, s1, c1) in units:
            nc.scalar.activation(d, s0, AF.Copy, scale=c0)
        for (d, s0, c0, s1, c1) in units:
            nc.vector.scalar_tensor_tensor(d, s1, c1, d, ALU.mult, ALU.add)
        self.spare, self.cur[bt] = self.cur[bt], self.spare

    # ---- final CRot ------------------------------------------------------

    def _final_crot(self, bt, D):
        """CR_{3,11}: ctrl=1 gets U, ctrl=0 identity copies; ping-pong."""
        nc = self.nc
        src, dst = self.cur[bt], self.spare
        # ctrl=0: plain copies
        for i, (t, c) in enumerate(OUTS):
            s = self.wsl(src, c, 0, t)
            d = self.wsl(dst, c, 0, t)
            e = CROT_COPY_ROT[i % len(CROT_COPY_ROT)]
            if e == "act":
                nc.scalar.copy(d, s)
            elif e == "dve":
                nc.vector.tensor_copy(out=d, in_=s)
            else:
                nc.gpsimd.tensor_copy(out=d, in_=s)
        # ctrl=1: gate with fixed broadcast coeffs
        chains = []
        for oi, (t, c) in enumerate(OUTS):
            dst_ap = self.wsl(dst, c, 1, t)
            terms = TERMS[(t, c)]
            srcs = [self.wsl(src, ci, 1, ti) for (_, ti, ci) in terms]
            planes = [pl for (pl, _, _) in terms]
            e = CROT_PLAN[oi % len(CROT_PLAN)]
            if e == "pe":
                self._pe_out(dst_ap, srcs, planes, D)
            else:
                chains.append((e, dst_ap, srcs,
                               [self.cco(pl) for pl in planes]))
        self._emit_chains(chains)
        self.spare, self.cur[bt] = self.cur[bt], self.spare

    # ---- observables -----------------------------------------------------

    def _observables(self, bt):
        """probs overwrite the re plane in place; im plane is scratch."""
        nc = self.nc
        buf = self.cur[bt]
        re = self.plane(buf, 0)
        im = self.plane(buf, 1)
        for h in range(4):
            sl = slice(h * 1024, (h + 1) * 1024)
            nc.scalar.activation(re[:, sl], re[:, sl], AF.Square)
            nc.scalar.activation(im[:, sl], im[:, sl], AF.Square)
            nc.vector.tensor_tensor(re[:, sl], re[:, sl], im[:, sl], ALU.add)
        w = DIM
        for q in range(N_QUBITS):
            h = w // 2
            lo, hi = re[:, 0:h], re[:, h:w]
            # (lo - hi) into f32 scratch, then reduce into ZT
            scr = self.SCR[bt % 2][:, 0:h]
            nc.vector.tensor_tensor(scr, lo, hi, ALU.subtract)
            nc.vector.tensor_reduce(out=self.ZT[bt][:, q:q + 1], in_=scr,
                                    op=ALU.add, axis=mybir.AxisListType.X)
            if q < N_QUBITS - 1:
                nc.vector.tensor_tensor(lo, lo, hi, ALU.add)
            w = h

    # ---- top level -------------------------------------------------------

    def _emit_circuit(self):
        for bt in range(NBT):
            self._load_prefix(bt)
        for k in range(KSTEPS):
            for bt in range(NBT):
                self._kron_step(k, bt)
        for g in range(N_GATES):
            q = g % 12
            for bt in range(NBT):
                self._fused_gate(g, q, bt)
        Dc = self._build_diags(lambda ci: self.cco(ci))
        for bt in range(NBT):
            self._final_crot(bt, Dc)
        for bt in range(NBT):
            self._observables(bt)


_PROG_CACHE = None


def _get_prog():
    global _PROG_CACHE
    if _PROG_CACHE is None:
        _PROG_CACHE = _Prog()
    return _PROG_CACHE


def _run(inputs, trace=False):
    x = np.asarray(inputs["x"], np.float32)
    fco, kco, pref, cco = _host_payload(
        x, inputs["q_params_rot"], inputs["q_params_enta"])
    # fco: [G,2,B,12] -> per-core tile [128, ((g*2+b)*12+ci)*4+bt]
    cco_tile = np.broadcast_to(cco.reshape(1, NCO), (128, NCO)).copy()
    ident = np.eye(128, dtype=np.float32)
    in_maps = []
    for core in range(N_CORES):
        lo = core * B_CORE
        f = fco[:, :, lo:lo + B_CORE, :]                  # [G,2,512,12]
        f = f.reshape(N_GATES, 2, NBT, 128, NCO)
        f = np.ascontiguousarray(np.transpose(f, (3, 0, 1, 4, 2)))
        k = kco[:, lo:lo + B_CORE, :].reshape(KSTEPS, NBT, 128, NCO)
        k = np.ascontiguousarray(np.transpose(k, (2, 0, 3, 1)))
        p = pref[lo:lo + B_CORE].reshape(NBT, 128, 64)    # complex
        pr = np.empty((128, NBT, 2, 64), np.float32)
        pr[:, :, 0, :] = np.moveaxis(p.real, 1, 0)
        pr[:, :, 1, :] = np.moveaxis(p.imag, 1, 0)
        in_maps.append({
            "fco": f.reshape(128, FCO_W),
            "diag": _host_diags(fco, lo),
            "kco": k.reshape(128, KCO_W),
            "pref": np.ascontiguousarray(pr.reshape(128, PREF_W)),
            "cco": cco_tile,
            "ident": ident,
        })
    prog = _get_prog()
    res = run_bass_kernel_spmd(prog.nc, in_maps, list(range(N_CORES)),
                               trace=trace)
    z = np.concatenate([res.results[c]["z"] for c in range(N_CORES)], axis=0)
    return z.astype(np.float32), res


def kernel(**inputs):
    z, _ = _run(inputs, trace=False)
    return z


# revision 28
# speedup vs baseline: 2.3840x; 1.0757x over previous
"""Trainium2 Bass kernel for a 12-qubit batched PennyLane-style circuit.

Fused formulation (validated in mirror.py):
  Circuit = prod_l [C_l P_l], P_l = tensor of per-sample 1q gates G_{l,q},
  C_l = ring of fixed CRots CR_{l,c} (ctrl c, target c+1 mod 12).
  Rewrite: C_l P_l = CR_{l,11} . [CR_{l,10} G_{l,11}] ... [CR_{l,0} G_{l,1}] . G_{l,0}
  so each CRot fuses with the per-sample gate on its target wire into a
  2-qubit gate that costs the same as a 1q gate (per-sample coefficients on
  both ctrl branches). The wrap CRot CR_{l,11} fuses into layer l+1's G on
  wire 0. Layer 0 (applied to |0..0>) becomes an MPS ladder product state:
  host builds the 64-amplitude prefix over wires 0-5, the device doubles it
  6 times (wires 6-11) with the ladder CRots folded in. Only CR_{3,11}
  remains as a real gate (emitted as a ping-pong gate with identity copies
  on the ctrl=0 half).

Distribution: pure data parallel, 4096 samples -> 8 cores x 512; per core
4 batch tiles of 128 samples (partitions). State = fp32 re/im planes,
batch on partitions, 5 ping-pong buffers (4 bt + 1 spare) so every gate
writes a fresh buffer: no copybacks, chains accumulate in the destination.

Engine split per gate site (gate, bt): ctrl=1 branch -> TensorE as diagonal
matmuls (per-sample coeffs on the diag) accumulating 4 terms in PSUM;
ctrl=0 branch -> 3 chains on DVE + 1 on GpSimd (4-term mult-add chains,
chain start on ScalarE); PSUM evictions + diag builds on ScalarE.
"""

import numpy as np

import concourse.bass as bass
import concourse.bacc as bacc
import concourse.mybir as mybir
from concourse.tile import TileContext
from concourse.bass_utils import run_bass_kernel_spmd

F32 = mybir.dt.float32
F32R = mybir.dt.float32r
F16 = mybir.dt.float16
STATE_DT = F16
ALU = mybir.AluOpType
AF = mybir.ActivationFunctionType

N_QUBITS = 12
N_LAYERS = 4
DIM = 4096
B_FULL = 4096
N_CORES = 8
B_CORE = B_FULL // N_CORES   # 512
NBT = B_CORE // 128          # 4

# coefficient plane order for a 2x2 complex gate [[a,b],[c,d]]
(ARE, AIM, MAIM, BRE, BIM, MBIM,
 CRE, CIM, MCIM, DRE, DIM_, MDIM) = range(12)
NCO = 12

N_GATES = 36                     # layers 1-3, 12 fused gates each
FCO_W = N_GATES * 2 * NCO * NBT  # 3456
KSTEPS = 6                       # device kron steps: wires 6..11
KCO_W = KSTEPS * NCO * NBT       # 288
PREF_W = NBT * 2 * 64            # 512

# per-output chain term tables: (t_out, comp_out) -> 4x (plane, t_in, comp_in)
TERMS = {
    (0, 0): ((ARE, 0, 0), (MAIM, 0, 1), (BRE, 1, 0), (MBIM, 1, 1)),
    (0, 1): ((AIM, 0, 0), (ARE, 0, 1), (BIM, 1, 0), (BRE, 1, 1)),
    (1, 0): ((CRE, 0, 0), (MCIM, 0, 1), (DRE, 1, 0), (MDIM, 1, 1)),
    (1, 1): ((CIM, 0, 0), (CRE, 0, 1), (DIM_, 1, 0), (DRE, 1, 1)),
}
OUTS = ((0, 0), (0, 1), (1, 0), (1, 1))

# ---------------------------------------------------------------------------
# engine plan knobs
# gpsimd can NOT run scalar_tensor_tensor (neuronxcc rejects it on Pool) and
# has ~us-scale per-op launch overhead on HW, so chains are DVE-only and
# gpsimd is unused. ScalarE has ~270ns fixed overhead per op, so diag
# matrices are prebuilt on the host and streamed in via (otherwise idle) DMA.
# branch -> full-width outputs on the PE diag-matmul path vs DVE chains
PE_OUTS = {1: ((0, 0), (0, 1), (1, 0), (1, 1)), 0: ((0, 0),)}
CHAIN_OUTS = {0: ((1, 0), (1, 1))}
HALF_OUTS = {0: ((0, 1),)}        # chunk 0 -> PE, chunk 1 -> DVE chain
# staged diag planes per site, in storage order
STAGE_PLANES = tuple((1, ci) for ci in range(NCO)) + \
    tuple((0, ci) for ci in (ARE, MAIM, BRE, MBIM, AIM, BIM))
NSTAGE = len(STAGE_PLANES)        # 16
N_SITES = N_GATES * NBT           # 144
DIAG_W = N_SITES * NSTAGE * 128   # staged diag dram cols (fp16)
EVICT_ROT = ("act",)              # psum eviction engines
DIAG_ROT = ("act",)               # diag build engines (final CRot only)
CROT_PLAN = ("pe", "dve", "pe", "dve")      # final CRot ctrl=1 outputs
CROT_COPY_ROT = ("act", "dve", "act", "dve")  # final CRot ctrl=0 copies

# ---------------------------------------------------------------------------
# host-side gate algebra
# ---------------------------------------------------------------------------


def _rz(t):
    e = np.exp(-0.5j * t)
    z = np.zeros_like(e)
    return np.stack([np.stack([e, z], -1), np.stack([z, np.conj(e)], -1)], -2)


def _ry(t):
    c = np.cos(t / 2).astype(np.complex128)
    s = np.sin(t / 2).astype(np.complex128)
    return np.stack([np.stack([c, -s], -1), np.stack([s, c], -1)], -2)


def _rx(t):
    c = np.cos(t / 2).astype(np.complex128)
    s = np.sin(t / 2).astype(np.complex128)
    return np.stack([np.stack([c, -1j * s], -1), np.stack([-1j * s, c], -1)], -2)


def _rot(phi, theta, omega):
    # PennyLane Rot = RZ(omega) @ RY(theta) @ RZ(phi)
    return _rz(omega) @ _ry(theta) @ _rz(phi)


def _coef_planes(g):
    """g: [..., 2, 2] complex -> [..., 12] float32 coefficient planes."""
    a, b = g[..., 0, 0], g[..., 0, 1]
    c, d = g[..., 1, 0], g[..., 1, 1]
    cols = [a.real, a.imag, -a.imag, b.real, b.imag, -b.imag,
            c.real, c.imag, -c.imag, d.real, d.imag, -d.imag]
    return np.stack(cols, -1).astype(np.float32)


def _host_gates(x, q_params_rot, q_params_enta):
    x = np.asarray(x, np.float64)
    pr = np.asarray(q_params_rot, np.float64)
    pe = np.asarray(q_params_enta, np.float64)
    enc = np.einsum('qbij,qbjk->qbik',
                    _ry(x[:, 3, :].T),
                    np.einsum('qbij,qbjk->qbik', _rz(x[:, 2, :].T),
                              np.einsum('qbij,qbjk->qbik',
                                        _rx(x[:, 1, :].T), _ry(x[:, 0, :].T))))
    rot = _rot(pr[..., 0], pr[..., 1], pr[..., 2])      # [L,Q,2,2]
    G = np.einsum('lqij,qbjk->lqbik', rot, enc)         # [L,Q,B,2,2]
    U = _rot(pe[..., 0], pe[..., 1], pe[..., 2])        # [L,Q,2,2]
    return G, U


def _host_payload(x, q_params_rot, q_params_enta):
    """Full-batch coefficient arrays: fco [B,3456-layout], kco, pref, cco."""
    B = x.shape[0]
    G, U = _host_gates(x, q_params_rot, q_params_enta)

    # fused gate branch matrices
    fco = np.empty((N_GATES, 2, B, NCO), np.float32)
    for g in range(N_GATES):
        l, q = 1 + g // 12, g % 12
        M0 = G[l, q]
        if q == 0:
            M1 = np.einsum('bij,jk->bik', G[l, 0], U[l - 1, 11])
        else:
            M1 = np.einsum('ij,bjk->bik', U[l, q - 1], G[l, q])
        fco[g, 0] = _coef_planes(M0)
        fco[g, 1] = _coef_planes(M1)

    # kron ladder steps q=6..11: chi_q(b)[j] as a 2x2 "matrix" M[b][j]
    kco = np.empty((KSTEPS, B, NCO), np.float32)
    for k in range(KSTEPS):
        q = 6 + k
        v0 = G[0, q, :, :, 0]                            # [B,2]
        v1 = np.einsum('ij,bj->bi', U[0, q - 1], v0)
        KM = np.empty((B, 2, 2), np.complex128)
        KM[:, 0, :] = v0
        KM[:, 1, :] = v1
        kco[k] = _coef_planes(KM)

    # 64-amplitude prefix over wires 0-5 (ladder CRots folded)
    pref = G[0, 0, :, :, 0]                              # [B,2]
    for q in range(1, 6):
        v0 = G[0, q, :, :, 0]
        v1 = np.einsum('ij,bj->bi', U[0, q - 1], v0)
        w = pref.shape[1]
        new = np.empty((B, 2 * w), np.complex128)
        nv = new.reshape(B, w // 2, 2, 2) if w > 1 else None
        if w == 1:
            raise AssertionError
        old = pref.reshape(B, w // 2, 2)
        for b in (0, 1):
            chi = v0 if b == 0 else v1
            for j in (0, 1):
                nv[:, :, b, j] = old[:, :, b] * chi[:, j][:, None]
        pref = new                                       # [B, 64]

    cco = _coef_planes(U[3, 11])                         # [12]
    return fco, kco, pref, cco


def _host_diags(fco, lo):
    """Staged diag matrices for one core: [128, N_SITES*NSTAGE*128] fp16.
    Site order: gate-major, bt-minor. Diag k of site (g,bt) holds
    coef-plane STAGE_PLANES[k] of that gate/branch on its diagonal."""
    d = np.zeros((128, N_SITES, NSTAGE, 128), np.float16)
    idx = np.arange(128)
    for g in range(N_GATES):
        for bt in range(NBT):
            s = g * NBT + bt
            sl = slice(lo + bt * 128, lo + (bt + 1) * 128)
            for k, (b, ci) in enumerate(STAGE_PLANES):
                d[idx, s, k, idx] = fco[g, b, sl, ci].astype(np.float16)
    return d.reshape(128, DIAG_W)


# ---------------------------------------------------------------------------
# bass program
# ---------------------------------------------------------------------------


class _Prog:
    def __init__(self):
        nc = bacc.Bacc("TRN2", target_bir_lowering=False, debug=False)
        self.nc = nc
        self.fco_d = nc.declare_dram_parameter("fco", [128, FCO_W], F32,
                                               isOutput=False)
        self.diag_d = nc.declare_dram_parameter("diag", [128, DIAG_W], F16,
                                                isOutput=False)
        self.kco_d = nc.declare_dram_parameter("kco", [128, KCO_W], F32,
                                               isOutput=False)
        self.pref_d = nc.declare_dram_parameter("pref", [128, PREF_W], F32,
                                                isOutput=False)
        self.cco_d = nc.declare_dram_parameter("cco", [128, NCO], F32,
                                               isOutput=False)
        self.idn_d = nc.declare_dram_parameter("ident", [128, 128], F32,
                                               isOutput=False)
        self.z_d = nc.declare_dram_parameter("z", [B_CORE, N_QUBITS], F32,
                                             isOutput=True)
        self._ectr = 0      # eviction engine rotation
        self._dctr = 0      # diag engine rotation
        self._cctr = 0      # chain plan rotation
        with TileContext(nc) as tc:
            self.tc = tc
            with tc.tile_pool(name="main", bufs=1) as pool, \
                    tc.tile_pool(name="dpool", bufs=14) as dpool, \
                    tc.tile_pool(name="dstage", bufs=6) as dstage, \
                    tc.tile_pool(name="psum", bufs=4, space="PSUM") as ppool:
                self.dpool = dpool
                self.dstage = dstage
                self.ppool = ppool
                # 5 ping-pong plane-pair buffers [re | im], each [128, 8192]
                self.BUF = [pool.tile([128, 2 * DIM], STATE_DT, name=f"st{i}",
                                      tag=f"st{i}") for i in range(5)]
                self.FC = pool.tile([128, FCO_W], F32, tag="fc")
                self.KC = pool.tile([128, KCO_W], F32, tag="kc")
                self.CC = pool.tile([128, NCO], F32, tag="cc")
                self.PS = pool.tile([128, PREF_W], F32, tag="prefs")
                self.I128 = pool.tile([128, 128], F32, tag="ident")
                self.ZT = [pool.tile([128, 16], F32, name=f"z{bt}",
                                     tag=f"z{bt}") for bt in range(NBT)]
                self.SCR = [pool.tile([128, 2048], F32, name=f"scr{i}",
                                      tag=f"scr{i}") for i in range(2)]
                self.cur = [0, 1, 2, 3]
                self.spare = 4

                nc.sync.dma_start(out=self.FC[:], in_=self.fco_d[:])
                nc.sync.dma_start(out=self.KC[:], in_=self.kco_d[:])
                nc.sync.dma_start(out=self.CC[:], in_=self.cco_d[:])
                nc.sync.dma_start(out=self.PS[:], in_=self.pref_d[:])
                nc.sync.dma_start(out=self.I128[:], in_=self.idn_d[:])

                self._emit_circuit()

                for bt in range(NBT):
                    nc.sync.dma_start(
                        out=self.z_d[bt * 128:(bt + 1) * 128, :],
                        in_=self.ZT[bt][:, 0:N_QUBITS])
        nc.compile()

    # ---- AP helpers -----------------------------------------------------

    def plane(self, buf, comp):
        return self.BUF[buf][:, comp * DIM:(comp + 1) * DIM]

    def fsl(self, buf, comp, q, b, t):
        """F-gate slice (wires q-1,q), q in 1..11: ctrl bit=b, target bit=t."""
        p = self.plane(buf, comp)
        if q == 11:
            v = p.rearrange("p (a c t) -> p a c t", c=2, t=2)
            return v[:, :, b, t]                     # [p, 1024] stride 4
        A = 1 << (q - 1)
        R = 1 << (11 - q)
        v = p.rearrange("p (a c t r) -> p a c t r", a=A, c=2, t=2, r=R)
        return v[:, :, b, t, :]                      # [p, A, R]

    def wsl(self, buf, comp, b, t):
        """Wrap-gate slice (wires 11,0): ctrl a11 (LSB)=b, target a0 (MSB)=t."""
        p = self.plane(buf, comp)
        v = p.rearrange("p (t a c) -> p t a c", t=2, c=2)
        return v[:, t, :, b]                         # [p, 1024] stride 2

    def fco(self, g, b, ci, bt):
        idx = ((g * 2 + b) * NCO + ci) * NBT + bt
        return self.FC[:, idx:idx + 1]

    def kco(self, k, ci, bt):
        idx = (k * NCO + ci) * NBT + bt
        return self.KC[:, idx:idx + 1]

    def cco(self, ci):
        return self.CC[:, ci:ci + 1]

    @staticmethod
    def _chunk(view, idx, csz):
        """csz-wide column chunk of an AP shaped [128, w] or [128, n, s]."""
        shp = view.shape[1:]
        if len(shp) == 1:
            return view[:, idx * csz:(idx + 1) * csz]
        n, s = shp
        if s >= csz:
            m = s // csz
            return view[:, idx // m, (idx % m) * csz:(idx % m + 1) * csz]
        na = csz // s
        return view[:, idx * na:(idx + 1) * na, :]

    def _eng(self, name):
        return {"dve": self.nc.vector, "gps": self.nc.gpsimd}[name]

    # ---- gate emission --------------------------------------------------

    def _build_diags(self, co):
        nc = self.nc
        D = {}
        for ci in range(NCO):
            d = self.dpool.tile([128, 128], STATE_DT, name="dg", tag="dg")
            e = DIAG_ROT[self._dctr % len(DIAG_ROT)]
            self._dctr += 1
            if e == "act":
                nc.scalar.activation(d[:], self.I128[:], AF.Copy,
                                     scale=co(ci))
            else:
                self._eng(e).tensor_scalar(d[:], self.I128[:], co(ci),
                                           None, ALU.mult)
            D[ci] = d
        return D

    def _pe_out(self, dst, srcs, planes, D):
        """One full-width (1024) output via TensorE diag matmuls into one
        wide PSUM tile (two 512 accumulation regions), one eviction."""
        nc = self.nc
        ps = self.ppool.tile([128, 1024], F32, name="ps", tag="ps")
        for k in range(4):
            for h in range(2):
                nc.tensor.matmul(out=ps[:, h * 512:(h + 1) * 512],
                                 lhsT=D[planes[k]][:],
                                 rhs=self._chunk(srcs[k], h, 512),
                                 start=(k == 0), stop=(k == 3))
        src = ps[:]
        if len(dst.shape) > 2:
            src = src.rearrange("p (a r) -> p a r", r=dst.shape[-1])
        nc.scalar.copy(dst, src)

    def _emit_site(self, site, sl_src, sl_dst, co):
        """Emit one fused-gate site: slices are dicts (b,t,comp)->AP.
        PE_OUTS outputs go to TensorE (host-staged diags, plane-grouped
        matmuls for lhsT reuse, one wide PSUM tile + eviction per output);
        CHAIN_OUTS outputs are 4-term DVE chains with ScalarE starts."""
        nc = self.nc
        chains = []     # (engine, dst, srcs[4], coefs[4])
        for b, outs in CHAIN_OUTS.items():
            for (t, c) in outs:
                terms = TERMS[(t, c)]
                chains.append(("dve", sl_dst[(b, t, c)],
                               [sl_src[(b, ti, ci)] for (_, ti, ci) in terms],
                               [co(b, pl) for (pl, _, _) in terms]))
        for b, outs in HALF_OUTS.items():
            for (t, c) in outs:
                terms = TERMS[(t, c)]
                chains.append((
                    "dve", self._chunk(sl_dst[(b, t, c)], 1, 512),
                    [self._chunk(sl_src[(b, ti, ci)], 1, 512)
                     for (_, ti, ci) in terms],
                    [co(b, pl) for (pl, _, _) in terms]))
        # chain starts (ScalarE) first so DVE can begin
        for (e, dst, srcs, coefs) in chains:
            nc.scalar.activation(dst, srcs[0], AF.Copy, scale=coefs[0])
        # staged diags for this site arrive by DMA
        stg = self.dstage.tile([128, NSTAGE * 128], F16, name="stg",
                               tag="stg")
        nc.sync.dma_start(
            out=stg[:],
            in_=self.diag_d[:, site * NSTAGE * 128:(site + 1) * NSTAGE * 128])
        # PE: one [128,1024] psum tile per output; matmuls grouped by plane
        jobs = {}   # (b, o) -> list of chunks
        for b, outs in PE_OUTS.items():
            for o in outs:
                jobs[(b, o)] = (0, 1)
        for b, outs in HALF_OUTS.items():
            for o in outs:
                jobs[(b, o)] = (0,)
        psums, nterm = {}, {}
        for key in jobs:
            psums[key] = self.ppool.tile([128, 1024], F32, name="ps",
                                         tag="ps")
            nterm[key] = [0, 0]
        for k, (b, ci) in enumerate(STAGE_PLANES):
            lhsT = stg[:, k * 128:(k + 1) * 128]
            for (bb, o), chunks in jobs.items():
                if bb != b:
                    continue
                for (pl, ti, cii) in TERMS[o]:
                    if pl != ci:
                        continue
                    src = sl_src[(b, ti, cii)]
                    ps = psums[(b, o)]
                    for h in chunks:
                        cnt = nterm[(b, o)]
                        nc.tensor.matmul(
                            out=ps[:, h * 512:(h + 1) * 512],
                            lhsT=lhsT, rhs=self._chunk(src, h, 512),
                            start=(cnt[h] == 0), stop=(cnt[h] == 3))
                        cnt[h] += 1
        for (b, o), chunks in jobs.items():
            full = len(chunks) == 2
            dst = sl_dst[(b, o[0], o[1])]
            if not full:
                dst = self._chunk(dst, 0, 512)
            src = psums[(b, o)][:, 0:1024 if full else 512]
            if len(dst.shape) > 2:
                src = src.rearrange("p (a r) -> p a r", r=dst.shape[-1])
            nc.scalar.copy(dst, src)
        # DVE accumulation rounds
        for k in range(1, 4):
            for (e, dst, srcs, coefs) in chains:
                self._eng(e).scalar_tensor_tensor(dst, srcs[k], coefs[k],
                                                  dst, ALU.mult, ALU.add)

    def _emit_chains(self, chains):
        """chains: list of (eng, dst, srcs[4], coefs[4]); starts on ScalarE,
        then STT accumulation interleaved round-robin per engine."""
        nc = self.nc
        for (e, dst, srcs, coefs) in chains:
            nc.scalar.activation(dst, srcs[0], AF.Copy, scale=coefs[0])
        for k in range(1, 4):
            for (e, dst, srcs, coefs) in chains:
                self._eng(e).scalar_tensor_tensor(dst, srcs[k], coefs[k],
                                                  dst, ALU.mult, ALU.add)

    def _fused_gate(self, g, q, bt):
        src, dst = self.cur[bt], self.spare
        sl_src, sl_dst = {}, {}
        for b in (0, 1):
            for t in (0, 1):
                for c in (0, 1):
                    if q == 0:
                        sl_src[(b, t, c)] = self.wsl(src, c, b, t)
                        sl_dst[(b, t, c)] = self.wsl(dst, c, b, t)
                    else:
                        sl_src[(b, t, c)] = self.fsl(src, c, q, b, t)
                        sl_dst[(b, t, c)] = self.fsl(dst, c, q, b, t)
        co = lambda b, ci: self.fco(g, b, ci, bt)
        self._emit_site(g * NBT + bt, sl_src, sl_dst, co)
        self.spare, self.cur[bt] = self.cur[bt], self.spare

    # ---- layer 0: prefix load + kron ladder ------------------------------

    def _load_prefix(self, bt):
        nc = self.nc
        dst = self.BUF[self.cur[bt]].rearrange("p (c n) -> p c n", c=2)
        src = self.PS[:, bt * 128:(bt + 1) * 128].rearrange(
            "p (c n) -> p c n", c=2)
        nc.scalar.copy(dst[:, :, 0:64], src)

    def _kron_step(self, k, bt):
        """Double width w -> 2w appending wire q=6+k, ladder CRot folded."""
        nc = self.nc
        w = 64 << k
        src, dst = self.cur[bt], self.spare
        units = []
        for b in (0, 1):
            for j in (0, 1):
                pl = 3 * (2 * b + j)     # re plane of entry [b][j]
                for comp in (0, 1):
                    old_re = self.plane(src, 0)[:, 0:w].rearrange(
                        "p (a pb) -> p a pb", pb=2)[:, :, b]
                    old_im = self.plane(src, 1)[:, 0:w].rearrange(
                        "p (a pb) -> p a pb", pb=2)[:, :, b]
                    d = self.plane(dst, comp)[:, 0:2 * w].rearrange(
                        "p (a pb j) -> p a pb j", pb=2, j=2)[:, :, b, j]
                    if comp == 0:
                        # re = old_re*chi_re + old_im*(-chi_im)
                        units.append((d, old_re, self.kco(k, pl, bt),
                                      old_im, self.kco(k, pl + 2, bt)))
                    else:
                        # im = old_re*chi_im + old_im*chi_re
                        units.append((d, old_re, self.kco(k, pl + 1, bt),
                                      old_im, self.kco(k, pl, bt)))
        for (d, s0, c0, s1, c1) in units:
            nc.scalar.activation(d, s0, AF.Copy, scale=c0)
        for (d, s0, c0, s1, c1) in units:
            nc.vector.scalar_tensor_tensor(d, s1, c1, d, ALU.mult, ALU.add)
        self.spare, self.cur[bt] = self.cur[bt], self.spare

    # ---- final CRot ------------------------------------------------------

    def _final_crot(self, bt, D):
        """CR_{3,11}: ctrl=1 gets U, ctrl=0 identity copies; ping-pong."""
        nc = self.nc
        src, dst = self.cur[bt], self.spare
        # ctrl=0: plain copies
        for i, (t, c) in enumerate(OUTS):
            s = self.wsl(src, c, 0, t)
            d = self.wsl(dst, c, 0, t)
            e = CROT_COPY_ROT[i % len(CROT_COPY_ROT)]
            if e == "act":
                nc.scalar.copy(d, s)
            elif e == "dve":
                nc.vector.tensor_copy(out=d, in_=s)
            else:
                nc.gpsimd.tensor_copy(out=d, in_=s)
        # ctrl=1: gate with fixed broadcast coeffs
        chains = []
        for oi, (t, c) in enumerate(OUTS):
            dst_ap = self.wsl(dst, c, 1, t)
            terms = TERMS[(t, c)]
            srcs = [self.wsl(src, ci, 1, ti) for (_, ti, ci) in terms]
            planes = [pl for (pl, _, _) in terms]
            e = CROT_PLAN[oi % len(CROT_PLAN)]
            if e == "pe":
                self._pe_out(dst_ap, srcs, planes, D)
            else:
                chains.append((e, dst_ap, srcs,
                               [self.cco(pl) for pl in planes]))
        self._emit_chains(chains)
        self.spare, self.cur[bt] = self.cur[bt], self.spare

    # ---- observables -----------------------------------------------------

    def _observables(self, bt):
        """probs overwrite the re plane in place; im plane is scratch."""
        nc = self.nc
        buf = self.cur[bt]
        re = self.plane(buf, 0)
        im = self.plane(buf, 1)
        for h in range(4):
            sl = slice(h * 1024, (h + 1) * 1024)
            nc.scalar.activation(re[:, sl], re[:, sl], AF.Square)
            nc.scalar.activation(im[:, sl], im[:, sl], AF.Square)
            nc.vector.tensor_tensor(re[:, sl], re[:, sl], im[:, sl], ALU.add)
        w = DIM
        for q in range(N_QUBITS):
            h = w // 2
            lo, hi = re[:, 0:h], re[:, h:w]
            # (lo - hi) into f32 scratch, then reduce into ZT
            scr = self.SCR[bt % 2][:, 0:h]
            nc.vector.tensor_tensor(scr, lo, hi, ALU.subtract)
            nc.vector.tensor_reduce(out=self.ZT[bt][:, q:q + 1], in_=scr,
                                    op=ALU.add, axis=mybir.AxisListType.X)
            if q < N_QUBITS - 1:
                nc.vector.tensor_tensor(lo, lo, hi, ALU.add)
            w = h

    # ---- top level -------------------------------------------------------

    def _emit_circuit(self):
        for bt in range(NBT):
            self._load_prefix(bt)
        for k in range(KSTEPS):
            for bt in range(NBT):
                self._kron_step(k, bt)
        for g in range(N_GATES):
            q = g % 12
            for bt in range(NBT):
                self._fused_gate(g, q, bt)
        Dc = self._build_diags(lambda ci: self.cco(ci))
        for bt in range(NBT):
            self._final_crot(bt, Dc)
        for bt in range(NBT):
            self._observables(bt)


_PROG_CACHE = None


def _get_prog():
    global _PROG_CACHE
    if _PROG_CACHE is None:
        _PROG_CACHE = _Prog()
    return _PROG_CACHE


def _run(inputs, trace=False):
    x = np.asarray(inputs["x"], np.float32)
    fco, kco, pref, cco = _host_payload(
        x, inputs["q_params_rot"], inputs["q_params_enta"])
    # fco: [G,2,B,12] -> per-core tile [128, ((g*2+b)*12+ci)*4+bt]
    cco_tile = np.broadcast_to(cco.reshape(1, NCO), (128, NCO)).copy()
    ident = np.eye(128, dtype=np.float32)
    in_maps = []
    for core in range(N_CORES):
        lo = core * B_CORE
        f = fco[:, :, lo:lo + B_CORE, :]                  # [G,2,512,12]
        f = f.reshape(N_GATES, 2, NBT, 128, NCO)
        f = np.ascontiguousarray(np.transpose(f, (3, 0, 1, 4, 2)))
        k = kco[:, lo:lo + B_CORE, :].reshape(KSTEPS, NBT, 128, NCO)
        k = np.ascontiguousarray(np.transpose(k, (2, 0, 3, 1)))
        p = pref[lo:lo + B_CORE].reshape(NBT, 128, 64)    # complex
        pr = np.empty((128, NBT, 2, 64), np.float32)
        pr[:, :, 0, :] = np.moveaxis(p.real, 1, 0)
        pr[:, :, 1, :] = np.moveaxis(p.imag, 1, 0)
        in_maps.append({
            "fco": f.reshape(128, FCO_W),
            "diag": _host_diags(fco, lo),
            "kco": k.reshape(128, KCO_W),
            "pref": np.ascontiguousarray(pr.reshape(128, PREF_W)),
            "cco": cco_tile,
            "ident": ident,
        })
    prog = _get_prog()
    res = run_bass_kernel_spmd(prog.nc, in_maps, list(range(N_CORES)),
                               trace=trace)
    z = np.concatenate([res.results[c]["z"] for c in range(N_CORES)], axis=0)
    return z.astype(np.float32), res


def kernel(**inputs):
    z, _ = _run(inputs, trace=False)
    return z
